# revision 1
# baseline (speedup 1.0000x reference)
"""Trainium2 8-core kernel for nn_Attention_70892730187933 (sparse multi-query attention).

Sharding: sequence-parallel over query rows. Core c owns rows {i : i % 8 == c},
as 2 blocks of 128 rows (block0 < 1024, block1 >= 1024). Causal trimming:
block0 needs key j-tiles 0..8, block1 needs 0..16 (key space padded to
17*128 = 2176 incl. 2 null cols). No collectives; host concatenates rows.

Activations stay transposed [feature, token] so every matmul contraction dim
lands on partitions with no on-device activation transposes. LayerNorm folds
into the Q projection (rank-1 correction); bias+masks are a packed additive
bf16 tensor added on TensorE via identity-matmul; exp runs on ScalarE straight
from PSUM; rowsums ride along as a ones-column in V.

Raw Block + explicit semaphores: this walrus build rejects instructions with
multiple attached sync waits, so Tile-generated sync cannot compile; every
cross-engine wait here is its own sequencer instruction. The builder plans all
four engine programs in one pass (semaphore counters known at plan time), then
emits them inside one Block.
"""

import sys
import numpy as np

sys.path.insert(0, "/opt/trn_rl_repo")

B, N, DIM, HEADS, DIM_HEAD, NUM_NULL = 1, 2048, 1024, 16, 64, 2
INNER = HEADS * DIM_HEAD
EPS = 1e-5
NCORES = 8
JT = 17
JPAD = JT * 128
NQ = 256
MASK_VAL = -30000.0
SH_JT = 9
NPAIR = HEADS // 2
EWP = SH_JT * 512 + (JT - SH_JT) * 256  # 6656: packed pair E/bias width
# psum chunk map: c0=jt0-2 (psA), c1=jt3-5 (psB), c2=jt6-8 (psA), c3=solo (psB)
CH_OF_JT = [0, 0, 0, 1, 1, 1, 2, 2, 2] + [3] * 8
CH_LEN = [1536, 1536, 1536, 2048]
CH_ECOL = [0, 1536, 3072, 4608]

_CACHE = {}


def _ecol(jt):
    """Column of j-tile jt in packed pair E/bias layout."""
    return jt * 512 if jt < SH_JT else SH_JT * 512 + (jt - SH_JT) * 256


def _ccol(jt):
    """Column of j-tile jt inside its psum chunk."""
    if jt < SH_JT:
        return (jt % 3) * 512
    return (jt - SH_JT) * 256


def _build_graph():
    from contextlib import ExitStack
    import concourse.bass as bass
    import concourse.mybir as mybir

    dt = mybir.dt
    F32, BF16 = dt.float32, dt.bfloat16
    AF = mybir.ActivationFunctionType
    OP = mybir.AluOpType
    AX = mybir.AxisListType
    nc = bass.Bass()

    d_in = {}
    for name, shape, ty in [
        ("xq", [NQ, DIM], F32), ("xtq", [DIM, NQ], F32), ("xt", [DIM, N], BF16),
        ("wq", [DIM, INNER], BF16), ("srow", [1, INNER], BF16),
        ("wkv", [DIM, 2 * DIM_HEAD], BF16), ("nkvt", [128, NUM_NULL], BF16),
        ("wout", [INNER, DIM], BF16), ("ibf", [128, 128], BF16),
        ("if32", [128, 128], F32), ("onesbf", [1, 128], BF16),
        ("onesf", [1, 128], F32), ("biasp", [NPAIR, 128, EWP], BF16),
    ]:
        d_in[name] = nc.declare_dram_parameter(name, shape, ty, isOutput=False)
    out_d = nc.declare_dram_parameter("out", [DIM, NQ], F32, isOutput=True)
    rs_dram = nc.dram_tensor("rs_scratch", [HEADS, NQ], F32)
    rc_dram = nc.dram_tensor("rc_scratch", [HEADS, NQ], BF16)

    ctx = ExitStack()
    sb = {}
    for name, shape, ty in [
        ("ibf", [128, 128], BF16), ("if32", [128, 128], F32),
        ("onesbf", [1, 128], BF16), ("onesf", [1, 128], F32),
        ("srow", [1, INNER], BF16), ("zb", [128, 1], F32), ("epsb", [128, 1], F32),
        ("wq", [128, 8 * INNER], BF16), ("wout", [128, 8 * DIM], BF16),
        ("wkv", [128, 8 * 128], BF16), ("xt", [128, 8 * N], BF16),
        ("xtq", [128, 8 * NQ], F32),
        ("xrow", [128, 2 * DIM], F32), ("xc", [128, DIM], F32),
        ("lns", [128, 12], F32),
        ("rsq_row", [1, NQ], F32), ("nmr_row", [1, NQ], F32),
        ("rsqb", [128, NQ], F32), ("negmurs", [1, NQ], BF16),
        ("xst", [128, 8 * NQ], BF16), ("qtmp", [128, 2 * NQ], BF16),
        ("kv", [128, JPAD], BF16), ("vsb", [128, JT * 65], BF16),
        ("e0", [128, EWP], BF16), ("e1", [128, EWP], BF16),
        ("b0", [128, EWP], BF16), ("b1", [128, EWP], BF16),
        ("oT", [64, HEADS * NQ], BF16), ("rs_row", [65, 2 * 512], F32),
        ("rs_all0", [8, NQ], F32), ("rs_all1", [8, NQ], F32),
        ("recip0", [8, NQ], F32), ("recip1", [8, NQ], F32),
        ("recip_bf0", [8, NQ], BF16), ("recip_bf1", [8, NQ], BF16),
        ("recipflat", [1, HEADS * NQ], BF16),
        ("oTn", [128, 8 * NQ], BF16), ("oTn_lo", [64, 8 * NQ], BF16),
        ("outsb", [128, 2 * NQ], F32),
    ] + [(f"qh{h}", [64, 2 * NQ], BF16) for h in range(NPAIR)]:
        sb[name] = ctx.enter_context(nc.sbuf_tensor("sb_" + name, shape, ty))

    qh = [sb[f"qh{h}"] for h in range(NPAIR)]
    esb = [sb["e0"], sb["e1"]]
    bsb = [sb["b0"], sb["b1"]]

    # PSUM: early tensors freed before head-loop tensors are allocated.
    early = ExitStack()
    kvp = [early.enter_context(nc.psum_tensor(f"kvp{i}", [128, 512], F32))
           for i in range(2)]
    qp = [early.enter_context(nc.psum_tensor(f"qp{i}", [128, NQ], F32))
          for i in range(2)]
    vp = [early.enter_context(nc.psum_tensor(f"vp{i}", [128, 64], BF16))
          for i in range(2)]
    stp = early.enter_context(nc.psum_tensor("stp", [1, 128], F32))
    rbp = early.enter_context(nc.psum_tensor("rbp", [128, NQ], F32))
    early.close()
    simA = ctx.enter_context(nc.psum_tensor("simA", [128, 1536], F32))
    simB = ctx.enter_context(nc.psum_tensor("simB", [128, 2048], F32))
    opp = ctx.enter_context(nc.psum_tensor("opp", [65, 512], F32))

    # ------- planner -------
    # DMA completions are out-of-order across queues, so each logical DMA
    # group gets its own semaphore; waits only use group-total (or
    # issue-gated) thresholds, which are unambiguous under any completion
    # order.
    plan = {"sync": [], "tensor": [], "vector": [], "scalar": []}
    DSEMS = ("dk", "dx", "dqx", "dsm", "dw", "dv",
             "db0", "db1", "dq", "dr", "dm", "dn", "do")
    cnt = {"p": 0, "v": 0, "s": 0, **{k: 0 for k in DSEMS}}
    SEM = {}

    def wait(eng, sem, thr):
        if thr > 0:
            plan[eng].append(lambda e, s=sem, t=thr: e.wait_ge(SEM[s], t))

    def dma(sem, out, in_):
        cnt[sem] += 16
        plan["sync"].append(
            lambda e, s=sem, o=out, i=in_: e.dma_start(out=o, in_=i)
            .then_inc(SEM[s], 16))
        return cnt[sem]

    def inc(eng, sem, fn):
        # DVE/ACT pipelines expose writes only after a drain; put the sem
        # update on the drain so both same-engine and cross-engine consumers
        # see committed data.
        cnt[sem] += 1
        if eng in ("vector", "scalar"):
            plan[eng].append(lambda e, f=fn: f(e))
            plan[eng].append(lambda e, s=sem: e.drain().then_inc(SEM[s], 1))
        else:
            plan[eng].append(lambda e, f=fn, s=sem: f(e).then_inc(SEM[s], 1))
        return cnt[sem]

    def run(eng, fn):
        plan[eng].append(fn)
        if eng in ("vector", "scalar"):
            plan[eng].append(lambda e: e.drain())

    def run_nodrain(eng, fn):
        plan[eng].append(fn)

    # ========== SYNC: const loads (kv path first so PE starts early) ======
    def dma8(sem, nm):
        return dma(sem, sb[nm][:].rearrange("p (c f) -> p c f", c=8),
                   d_in[nm][:].rearrange("(c p) f -> p c f", c=8))

    dma8("dk", "wkv")
    dma8("dk", "xt")
    d_kv = cnt["dk"]                      # 32
    d_xq = dma("dx", sb["xrow"][:].rearrange("p (t f) -> p t f", t=2),
               d_in["xq"][:].rearrange("(t p) f -> p t f", t=2))
    for nm in ("ibf", "if32", "onesbf", "onesf", "srow"):
        dma("dsm", sb[nm][:], d_in[nm][:])
    d_sm0 = dma("dsm", sb["kv"][:, 0:NUM_NULL], d_in["nkvt"][:])
    dma8("dqx", "wq")
    dma8("dqx", "xtq")
    d_qx = cnt["dqx"]                     # 32
    for p in range(2):
        dma(f"db{p}", bsb[p][:], d_in["biasp"][p])
    d_w = dma8("dw", "wout")

    # ========== VECTOR: memsets ==========
    run("vector", lambda e: e.memset(sb["zb"][:], 0.0))
    run("vector", lambda e: e.memset(sb["epsb"][:], EPS))
    run("vector", lambda e: e.memset(sb["vsb"][:], 1.0))
    v_memset = inc("vector", "v",
                   lambda e: e.memset(sb["kv"][:, NUM_NULL + N:JPAD], 0.0))

    # ========== LN stats: lns cols t*6 + {0 negmu, 1 ssq, 2 lnv, 3 rsqc, 4 nmrc}
    v_center = [0, 0]
    s_sq = [0, 0]
    s_rsqc = [0, 0]
    v_nmrc = [0, 0]
    for t in range(2):
        c0 = t * 6
        negmu = sb["lns"][:, c0:c0 + 1]
        if t == 0:
            wait("vector", "dx", d_xq)
        if t == 1:
            wait("vector", "s", s_sq[0])  # xc reuse
        run("vector", lambda e, t=t, negmu=negmu: e.tensor_reduce(
            out=negmu, in_=sb["xrow"][:, t * DIM:(t + 1) * DIM],
            axis=AX.X, op=OP.add, negate=True))
        run("vector", lambda e, negmu=negmu: e.tensor_scalar_mul(
            out=negmu, in0=negmu, scalar1=1.0 / DIM))
        v_center[t] = inc("vector", "v", lambda e, t=t, negmu=negmu:
                          e.tensor_scalar_add(
                              out=sb["xc"][:],
                              in0=sb["xrow"][:, t * DIM:(t + 1) * DIM],
                              scalar1=negmu))
        # scalar chain for this t
        if t == 0:
            wait("scalar", "v", v_memset)
        wait("scalar", "v", v_center[t])
        ssq = sb["lns"][:, c0 + 1:c0 + 2]
        lnv = sb["lns"][:, c0 + 2:c0 + 3]
        rsqc = sb["lns"][:, c0 + 3:c0 + 4]
        s_sq[t] = inc("scalar", "s", lambda e, t=t, ssq=ssq: e.activation(
            out=sb["xrow"][:, t * DIM:(t + 1) * DIM], in_=sb["xc"][:],
            func=AF.Square, bias=sb["zb"][:], accum_out=ssq))
        run("scalar", lambda e, ssq=ssq, lnv=lnv: e.activation(
            out=lnv, in_=ssq, func=AF.Ln, scale=1.0 / DIM, bias=sb["epsb"][:]))
        s_rsqc[t] = inc("scalar", "s", lambda e, lnv=lnv, rsqc=rsqc: e.activation(
            out=rsqc, in_=lnv, func=AF.Exp, scale=-0.5, bias=sb["zb"][:]))
        wait("vector", "s", s_rsqc[t])
        v_nmrc[t] = inc("vector", "v", lambda e, c0=c0: e.tensor_tensor(
            out=sb["lns"][:, c0 + 4:c0 + 5], in0=sb["lns"][:, c0:c0 + 1],
            in1=sb["lns"][:, c0 + 3:c0 + 4], op=OP.mult))

    # ========== TENSOR: kv matmuls (kvp double-buffered) ==========
    wait("tensor", "dk", d_kv)
    p_kvchunk = [0] * 4
    s_kvevac = [0] * 4
    for ch in range(4):
        pb = kvp[ch % 2]
        if ch >= 2:
            wait("tensor", "s", s_kvevac[ch - 2])
        for ct in range(8):
            fn = lambda e, pb=pb, ch=ch, ct=ct: e.matmul(
                pb[:], sb["wkv"][:, ct * 128:(ct + 1) * 128],
                sb["xt"][:, ct * N + ch * 512:ct * N + (ch + 1) * 512],
                start=(ct == 0), stop=(ct == 7))
            if ct == 7:
                p_kvchunk[ch] = inc("tensor", "p", fn)
            else:
                run("tensor", fn)
        wait("scalar", "p", p_kvchunk[ch])
        s_kvevac[ch] = inc("scalar", "s", lambda e, pb=pb, ch=ch: e.activation(
            out=sb["kv"][:, NUM_NULL + ch * 512:NUM_NULL + (ch + 1) * 512],
            in_=pb[:], func=AF.Copy))

    # ========== TENSOR: stats transposes + rsqb broadcast ==========
    v_statrow = [[0, 0], [0, 0]]
    wait("tensor", "dsm", d_sm0)  # if32/onesf/ibf/srow/nkvt loaded
    for t in range(2):
        c0 = t * 6
        wait("tensor", "s", s_rsqc[t])
        if t == 1:
            wait("tensor", "v", v_statrow[0][1])  # stp reuse
        pst = inc("tensor", "p", lambda e, c0=c0: e.transpose(
            stp[:], sb["lns"][:, c0 + 3:c0 + 4], sb["if32"][:]))
        wait("vector", "p", pst)
        v_statrow[t][0] = inc("vector", "v", lambda e, t=t: e.tensor_copy(
            sb["rsq_row"][0:1, t * 128:(t + 1) * 128], stp[:]))
        wait("tensor", "v", v_statrow[t][0])
        wait("tensor", "v", v_nmrc[t])
        pst2 = inc("tensor", "p", lambda e, c0=c0: e.transpose(
            stp[:], sb["lns"][:, c0 + 4:c0 + 5], sb["if32"][:]))
        wait("vector", "p", pst2)
        v_statrow[t][1] = inc("vector", "v", lambda e, t=t: e.tensor_copy(
            sb["nmr_row"][0:1, t * 128:(t + 1) * 128], stp[:]))

    wait("tensor", "v", v_statrow[1][0])
    p_rsqb = inc("tensor", "p", lambda e: e.matmul(
        rbp[:], sb["onesf"][0:1, :], sb["rsq_row"][0:1, :], start=True, stop=True))
    wait("vector", "p", p_rsqb)
    wait("vector", "dqx", d_qx)
    run("vector", lambda e: e.tensor_copy(sb["rsqb"][:], rbp[:]))
    v_negmurs = inc("vector", "v",
                    lambda e: e.tensor_copy(sb["negmurs"][:], sb["nmr_row"][0:1, :]))
    for ct in range(8):
        fn = lambda e, ct=ct: e.tensor_tensor(
            out=sb["xst"][:, ct * NQ:(ct + 1) * NQ],
            in0=sb["xtq"][:, ct * NQ:(ct + 1) * NQ], in1=sb["rsqb"][:], op=OP.mult)
        if ct == 7:
            v_xst = inc("vector", "v", fn)
        else:
            run("vector", fn)

    # ========== TENSOR: v transposes (vp double-buffered) ==========
    p_vt = [0] * JT
    v_vcopy = [0] * JT
    for jt in range(JT):
        pb = vp[jt % 2]
        ch_hi = min(3, ((jt + 1) * 128 - 1 - NUM_NULL) // 512)
        wait("tensor", "s", s_kvevac[ch_hi])
        if jt == JT - 1:
            wait("tensor", "v", v_memset)
        if jt >= 2:
            wait("tensor", "v", v_vcopy[jt - 2])
        p_vt[jt] = inc("tensor", "p", lambda e, pb=pb, jt=jt: e.transpose(
            pb[:], sb["kv"][64:128, jt * 128:(jt + 1) * 128],
            sb["ibf"][64:128, 64:128]))
        wait("vector", "p", p_vt[jt])
        v_vcopy[jt] = inc("vector", "v", lambda e, pb=pb, jt=jt: e.tensor_copy(
            sb["vsb"][:, jt * 65:jt * 65 + 64], pb[:]))
    v_vsb = v_vcopy[JT - 1]

    # ========== TENSOR: q projection (qp double-buffered) ==========
    wait("tensor", "v", v_xst)
    p_q = [0] * 8
    v_qtmp = [0] * 8
    d_qodd = [0] * 8
    for dtile in range(8):
        pb = qp[dtile % 2]
        if dtile >= 2:
            wait("tensor", "v", v_qtmp[dtile - 2])
        for ct in range(8):
            run("tensor", lambda e, pb=pb, dtile=dtile, ct=ct: e.matmul(
                pb[:],
                sb["wq"][:, ct * INNER + dtile * 128:ct * INNER + (dtile + 1) * 128],
                sb["xst"][:, ct * NQ:(ct + 1) * NQ],
                start=(ct == 0), stop=False))
        p_q[dtile] = inc("tensor", "p", lambda e, pb=pb, dtile=dtile: e.matmul(
            pb[:], sb["srow"][0:1, dtile * 128:(dtile + 1) * 128],
            sb["negmurs"][:], start=False, stop=True))
        wait("vector", "p", p_q[dtile])
        run("vector", lambda e, pb=pb, dtile=dtile: e.tensor_copy(
            qh[dtile][0:64, :].rearrange("a (b h c) -> a b h c", b=2, h=2)[:, :, 0, :],
            pb[0:64, :].rearrange("a (b c) -> a b c", b=2)))
        slot = dtile % 2
        if dtile >= 2:
            wait("vector", "dq", 16 * dtile)  # qtmp slot reuse (all issued)
        v_qtmp[dtile] = inc("vector", "v", lambda e, pb=pb, slot=slot:
                            e.tensor_copy(
                                sb["qtmp"][64:128, slot * NQ:(slot + 1) * NQ],
                                pb[64:128, :]))
        wait("sync", "v", v_qtmp[dtile])
        d_qodd[dtile] = dma(
            "dq",
            qh[dtile][0:64, :].rearrange("a (b h c) -> a b h c", b=2, h=2)[:, :, 1, :],
            sb["qtmp"][64:128, slot * NQ:(slot + 1) * NQ]
            .rearrange("a (b c) -> a b c", b=2))

    # ========== PAIR LOOP (2 heads per matmul: shared K/V stationary) ====
    v_pre_heads = cnt["v"]
    s_exp = [[0] * 4 for _ in range(NPAIR)]
    p_simc = [[0] * 4 for _ in range(NPAIR)]
    p_odone = [0] * NPAIR
    v_oevac = [0] * NPAIR
    d_rsexp = [0] * NPAIR

    v_oTnmult = [0] * HEADS
    d_oTn = [0] * HEADS
    norm_mflat = [0, 0]

    def norm_half(half):
        # half 0: heads 0..7 (pairs 0..3), half 1: heads 8..15
        h0, h1 = half * 8, half * 8 + 8
        p0 = half * 4
        ra, rc_, rb = (sb[f"rs_all{half}"], sb[f"recip{half}"],
                       sb[f"recip_bf{half}"])
        wait("sync", "dr", 16 * (p0 + 4))
        dma("dm", ra[:], rs_dram[h0:h1, :])
        m_imp = cnt["dm"]
        wait("vector", "dm", m_imp)
        run("vector", lambda e, ra=ra, rc_=rc_: e.reciprocal(
            out=rc_[:], in_=ra[:]))
        v_rb = inc("vector", "v", lambda e, rc_=rc_, rb=rb: e.tensor_copy(
            rb[:], rc_[:]))
        wait("sync", "v", v_rb)
        dma("dm", rc_dram[h0:h1, :], rb[:])
        wait("sync", "dm", cnt["dm"])
        dma("dm", sb["recipflat"][0:1, h0 * NQ:h1 * NQ],
            rc_dram[h0:h1, :].rearrange("h q -> (h q)")
            .rearrange("(o f) -> o f", o=1))
        norm_mflat[half] = cnt["dm"]

    wait("tensor", "s", s_kvevac[3])
    wait("tensor", "v", v_pre_heads)  # early-psum reuse guard
    wait("tensor", "dq", 16 * 8)      # all odd-half q DMAs done
    for p in range(NPAIR):
        eh = esb[p % 2]
        bh = bsb[p % 2]
        qpair = qh[p][0:64, :]
        qsolo = qh[p][0:64, NQ:2 * NQ]  # [h0 b1 | h1 b1], contiguous
        wait("tensor", f"db{p % 2}", 16 * (p // 2 + 1))

        for ck in range(4):
            ps = simA if ck in (0, 2) else simB
            jts = [jt for jt in range(JT) if CH_OF_JT[jt] == ck]
            # psum chunk reuse guards
            if p >= 1:
                wait("tensor", "s", s_exp[p - 1][{0: 2, 1: 3, 2: 0, 3: 1}[ck]])
            if ck >= 2:
                wait("tensor", "s", s_exp[p][ck - 2])
            for jt in jts:
                w = 512 if jt < SH_JT else 256
                rhs = qpair if jt < SH_JT else qsolo
                run("tensor", lambda e, ps=ps, jt=jt, r=rhs, w=w: e.matmul(
                    ps[:, _ccol(jt):_ccol(jt) + w],
                    sb["kv"][0:64, jt * 128:(jt + 1) * 128], r,
                    start=True, stop=False))
                fn2 = lambda e, ps=ps, jt=jt, bh=bh, w=w: e.matmul(
                    ps[:, _ccol(jt):_ccol(jt) + w],
                    sb["ibf"][:], bh[:, _ecol(jt):_ecol(jt) + w],
                    start=False, stop=True)
                if jt == jts[-1]:
                    p_simc[p][ck] = inc("tensor", "p", fn2)
                else:
                    run("tensor", fn2)
            # SCALAR: exp for this chunk (overlaps PE's next chunk)
            wait("scalar", "p", p_simc[p][ck])
            if ck == 0 and p >= 2:
                wait("scalar", "p", p_odone[p - 2])  # E slot reuse
            ln = CH_LEN[ck]
            s_exp[p][ck] = inc("scalar", "s", lambda e, ps=ps, ck=ck, ln=ln,
                               eh=eh: e.activation(
                                   out=eh[:, CH_ECOL[ck]:CH_ECOL[ck] + ln],
                                   in_=ps[:, 0:ln], func=AF.Exp, bias=sb["zb"][:]))

        # TENSOR: o matmuls (17-matmul accumulation group into opp [65, 512])
        osolo_out = opp[0:65, NQ:2 * NQ]
        if p == 0:
            wait("tensor", "v", v_vsb)
        if p >= 1:
            wait("tensor", "v", v_oevac[p - 1])  # opp reuse
        for jt in range(JT):
            ck = CH_OF_JT[jt]
            if jt == 0 or _ccol(jt) == 0 or (ck == 3 and jt == SH_JT):
                wait("tensor", "s", s_exp[p][ck])
            if jt < SH_JT:
                run("tensor", lambda e, jt=jt, eh=eh: e.matmul(
                    opp[:], sb["vsb"][:, jt * 65:jt * 65 + 65],
                    eh[:, _ecol(jt):_ecol(jt) + 512],
                    start=(jt == 0), stop=False))
            else:
                fn = lambda e, jt=jt, eh=eh: e.matmul(
                    osolo_out, sb["vsb"][:, jt * 65:jt * 65 + 65],
                    eh[:, _ecol(jt):_ecol(jt) + 256],
                    start=False, stop=(jt == JT - 1))
                if jt == JT - 1:
                    p_odone[p] = inc("tensor", "p", fn)
                else:
                    run("tensor", fn)

        # VECTOR: evacuate o rows + rowsum row (rs_row slot p%2)
        wait("vector", "p", p_odone[p])
        run("vector", lambda e, p=p: e.tensor_copy(
            sb["oT"][0:64, p * 512:(p + 1) * 512]
            .rearrange("a (h b c) -> a h b c", h=2, b=2),
            opp[0:64, :].rearrange("a (b h c) -> a h b c", b=2, h=2)))
        rslot = p % 2
        if p >= 2:
            wait("vector", "dr", 16 * p)  # rs_row slot reuse (all issued)
        v_oevac[p] = inc("vector", "v", lambda e, rslot=rslot: e.tensor_copy(
            sb["rs_row"][64:65, rslot * 512:(rslot + 1) * 512]
            .rearrange("a (h b c) -> a h b c", h=2, b=2),
            opp[64:65, :].rearrange("a (b h c) -> a h b c", b=2, h=2)))
        wait("sync", "v", v_oevac[p])
        d_rsexp[p] = dma("dr", rs_dram[2 * p:2 * p + 2, :]
                         .rearrange("h q -> (h q)").rearrange("(o f) -> o f", o=1),
                         sb["rs_row"][64:65, rslot * 512:(rslot + 1) * 512])

        # SYNC: bias prefetch for pair p+2 (slot free once sim of p done)
        if p + 2 < NPAIR:
            wait("sync", "p", p_simc[p][3])
            dma(f"db{(p + 2) % 2}", bsb[(p + 2) % 2][:], d_in["biasp"][p + 2])
        if p == NPAIR - 3:
            norm_half(0)
        if p == NPAIR - 1:
            norm_half(1)

    # (normalization planned inside the pair loop as two overlapped halves)
    # ---- per-head recip broadcast (PE) + normalize-multiply (DVE) ----
    wait("tensor", "dm", norm_mflat[1])
    wait("tensor", "v", v_oevac[NPAIR - 1])
    for h in range(HEADS):
        if h >= 1:
            wait("tensor", "v", v_oTnmult[h - 1])  # opp reuse
        pb = inc("tensor", "p", lambda e, h=h: e.matmul(
            opp[0:64, 0:NQ], sb["onesbf"][0:1, 0:64],
            sb["recipflat"][0:1, h * NQ:(h + 1) * NQ], start=True, stop=True))
        wait("vector", "p", pb)
        if h % 2 == 0:
            dst = sb["oTn"][0:64, (h // 2) * NQ:(h // 2 + 1) * NQ]
        else:
            dst = sb["oTn_lo"][0:64, (h // 2) * NQ:(h // 2 + 1) * NQ]
        v_oTnmult[h] = inc("vector", "v", lambda e, h=h, dst=dst: e.tensor_tensor(
            out=dst, in0=opp[0:64, 0:NQ], in1=sb["oT"][0:64, h * NQ:(h + 1) * NQ],
            op=OP.mult))
        if h % 2 == 1:
            wait("sync", "v", v_oTnmult[h])
            d_oTn[h] = dma("dn", sb["oTn"][64:128, (h // 2) * NQ:(h // 2 + 1) * NQ],
                           sb["oTn_lo"][0:64, (h // 2) * NQ:(h // 2 + 1) * NQ])

    # ========== output projection (accumulators alternate in simA banks) ====
    wait("tensor", "v", v_oTnmult[HEADS - 1])
    wait("tensor", "dn", 16 * 8)
    wait("tensor", "dw", d_w)
    s_outevac = [0] * 8
    d_out = [0] * 8
    for et in range(8):
        reg = simA[:, 0:NQ] if et % 2 == 0 else simA[:, 512:512 + NQ]
        if et >= 2:
            wait("tensor", "s", s_outevac[et - 2])
        p_wout = 0
        for hdt in range(8):
            fn = lambda e, et=et, hdt=hdt, reg=reg: e.matmul(
                reg, sb["wout"][:, hdt * DIM + et * 128:hdt * DIM + (et + 1) * 128],
                sb["oTn"][:, hdt * NQ:(hdt + 1) * NQ],
                start=(hdt == 0), stop=(hdt == 7))
            if hdt == 7:
                p_wout = inc("tensor", "p", fn)
            else:
                run("tensor", fn)
        wait("scalar", "p", p_wout)
        oslot = et % 2
        if et >= 2:
            wait("scalar", "do", 16 * et)  # outsb slot reuse (all issued)
        s_outevac[et] = inc("scalar", "s", lambda e, oslot=oslot, reg=reg:
                            e.activation(
                                out=sb["outsb"][:, oslot * NQ:(oslot + 1) * NQ],
                                in_=reg, func=AF.Copy))
        wait("sync", "s", s_outevac[et])
        d_out[et] = dma("do", out_d[et * 128:(et + 1) * 128, :],
                        sb["outsb"][:, oslot * NQ:(oslot + 1) * NQ])

    # ========== emit ==========
    from contextlib import ExitStack as _ES
    semctx = _ES()
    for k in ("p", "v", "s") + DSEMS:
        SEM[k] = semctx.enter_context(nc.semaphore(f"sem_{k}"))
    with semctx:
        with nc.Block() as block:
            @block.sync
            def _(e):
                for fn in plan["sync"]:
                    fn(e)

            @block.tensor
            def _(e):
                for fn in plan["tensor"]:
                    fn(e)

            @block.vector
            def _(e):
                for fn in plan["vector"]:
                    fn(e)

            @block.scalar
            def _(e):
                for fn in plan["scalar"]:
                    fn(e)
    ctx.close()
    return nc


def _prep_inputs(x, attn_bias, Wq, Wkv, null_kv, Wout, gamma, mask):
    from ml_dtypes import bfloat16
    x = np.asarray(x, np.float32)[0]            # [N, DIM]
    attn_bias = np.asarray(attn_bias, np.float32)[0]  # [H, N, N]
    Wq = np.asarray(Wq, np.float32)
    Wkv = np.asarray(Wkv, np.float32)
    null_kv = np.asarray(null_kv, np.float32)
    Wout = np.asarray(Wout, np.float32)
    gamma = np.asarray(gamma, np.float32)
    mask = np.asarray(mask, bool)[0]            # [N]

    scale = DIM_HEAD ** -0.5
    wq_eff = (gamma[:, None] * Wq * scale).astype(np.float32)
    srow = wq_eff.sum(axis=0, keepdims=True)
    xt = np.ascontiguousarray(x.T)
    nkvt = np.zeros((128, NUM_NULL), np.float32)
    nkvt[0:DIM_HEAD, :] = null_kv[0].T
    nkvt[64:64 + DIM_HEAD, :] = null_kv[1].T
    I128 = np.eye(128, dtype=np.float32)
    ones = np.ones((1, 128), np.float32)

    jpad = np.arange(JPAD)
    jvalid = np.zeros(JPAD, bool)
    jvalid[:NUM_NULL] = True
    jvalid[NUM_NULL:NUM_NULL + N] = mask
    key_of_j = jpad - NUM_NULL

    in_maps = []
    idx_all = []
    for c in range(NCORES):
        idx = np.concatenate([np.arange(c, 1024, 8), np.arange(1024 + c, 2048, 8)])
        idx_all.append(idx)
        allow = jvalid[None, :] & (key_of_j[None, :] <= idx[:, None])  # [NQ, JPAD]
        allow[:, :NUM_NULL] = True
        ab = np.zeros((HEADS, JPAD, NQ), np.float32)
        ab[:, NUM_NULL:NUM_NULL + N, :] = attn_bias[:, idx, :].transpose(0, 2, 1)
        bt = np.where(allow.T[None], ab, MASK_VAL)
        # pack per head-PAIR: shared jt -> [h0 (256) | h1 (256)]; solo jt ->
        # [h0 block1 (128) | h1 block1 (128)]
        pk = np.empty((HEADS // 2, 128, EWP), np.float32)
        for jt in range(SH_JT):
            c0 = jt * 512
            tile = bt[:, jt * 128:(jt + 1) * 128, :]        # [H, 128, 256]
            pk[:, :, c0:c0 + 128] = tile[0::2, :, 0:128]         # h0 b0
            pk[:, :, c0 + 128:c0 + 256] = tile[1::2, :, 0:128]   # h1 b0
            pk[:, :, c0 + 256:c0 + 384] = tile[0::2, :, 128:256]  # h0 b1
            pk[:, :, c0 + 384:c0 + 512] = tile[1::2, :, 128:256]  # h1 b1
        for jt in range(SH_JT, JT):
            c0 = SH_JT * 512 + (jt - SH_JT) * 256
            tile = bt[:, jt * 128:(jt + 1) * 128, 128:256]  # [H, 128, 128]
            pk[:, :, c0:c0 + 128] = tile[0::2]
            pk[:, :, c0 + 128:c0 + 256] = tile[1::2]
        in_maps.append({
            "xq": np.ascontiguousarray(x[idx]),
            "xtq": np.ascontiguousarray(xt[:, idx]),
            "xt": xt.astype(bfloat16),
            "wq": wq_eff.astype(bfloat16),
            "srow": srow.astype(bfloat16),
            "wkv": Wkv.astype(bfloat16),
            "nkvt": nkvt.astype(bfloat16),
            "wout": Wout.astype(bfloat16),
            "ibf": I128.astype(bfloat16),
            "if32": I128,
            "onesbf": ones.astype(bfloat16),
            "onesf": ones,
            "biasp": pk.astype(bfloat16),
        })
    return in_maps, idx_all


def _run(inputs, trace=False):
    from concourse.bass_utils import run_bass_kernel_spmd
    if "nc" not in _CACHE:
        _CACHE["nc"] = _build_graph()
    nc = _CACHE["nc"]
    in_maps, idx_all = _prep_inputs(**inputs)
    res = run_bass_kernel_spmd(nc, in_maps, list(range(NCORES)), trace=trace)
    out = np.zeros((B, N, DIM), np.float32)
    for c in range(NCORES):
        out[0, idx_all[c], :] = res.results[c]["out"].T
    return out, res


def kernel(**inputs):
    out, _ = _run(inputs, trace=False)
    return out



# revision 9
# speedup vs baseline: 1.5476x; 1.5476x over previous
"""Trainium2 8-core kernel for nn_Attention_70892730187933 (sparse multi-query attention).

Sharding: sequence-parallel over query rows. Core c owns rows {i : i % 8 == c},
as 2 blocks of 128 rows (block0 < 1024, block1 >= 1024). Key space padded to
17*128 = 2176 (incl. 2 null cols). No collectives; host concatenates rows.

v2 design vs baseline:
- Causal q-column trimming: for key tile jt, only q columns [S_jt, 256) per
  head can attend (S_jt = max(0, 16*jt-1), worst-case over cores); per-pair
  sim width drops 6656 -> 4384. q is packed head-major so each tile is one
  contiguous per-head slice.
- Bias is applied multiplicatively: host packs exp(bias) (0 where masked) and
  DVE multiplies it into exp(sim) at bf16 2x rate. This removes the
  identity-matmul bias adds (half of all sim PE work) entirely.
- Softmax normalization is inline: rowsums ride as a ones-column in V; DVE
  takes reciprocals straight from the PSUM rowsum row (no DRAM roundtrips)
  and per-head normalization overlaps the pair loop.
- DMA issue order prioritizes the critical path: consts, xq/xtq (LN chain),
  wkv + xt (kv chain, split in 4 column slices gating kv matmul chunks), wq,
  then bias tiles / wout.

Raw Block + explicit semaphores (this walrus build rejects multi-wait
instructions); the planner records semaphore counter targets at plan time,
then emits all four engine programs inside one Block.
"""

import sys
import numpy as np

sys.path.insert(0, "/opt/trn_rl_repo")

B, N, DIM, HEADS, DIM_HEAD, NUM_NULL = 1, 2048, 1024, 16, 64, 2
INNER = HEADS * DIM_HEAD
EPS = 1e-5
NCORES = 8
JT = 17
JPAD = JT * 128
NQ = 256
MASK_VAL = -30000.0
NPAIR = HEADS // 2

# per-head q-col start for key tile jt (worst case over cores => widest)
S_JT = [max(0, 16 * jt - 1) for jt in range(JT)]
W_JT = [256 - s for s in S_JT]          # per-head width
TW = [2 * w for w in W_JT]              # per-pair tile width (h0|h1 packed)
# E/bias pair layout: [h0 tiles packed (EW1) | h1 tiles packed (EW1)]
ECOL1 = [0] * JT
for _jt in range(1, JT):
    ECOL1[_jt] = ECOL1[_jt - 1] + W_JT[_jt - 1]
EW1 = ECOL1[-1] + W_JT[-1]              # 2192 per-head packed width
EW = 2 * EW1                            # 4384 packed pair E/bias width

# sim psum chunks: tiles packed into regions A(<=1536), B(<=1536), A(<=1536)
# region layout per chunk: [h0 tiles (L/2) | h1 tiles (L/2)]
CHUNKS = [[0, 1, 2], [3, 4, 5, 6], list(range(7, JT))]
CH_LEN = [sum(TW[j] for j in ck) for ck in CHUNKS]    # 1444, 1480, 1460
CH_E1 = [ECOL1[ck[0]] for ck in CHUNKS]               # per-head col offset
NCK = len(CHUNKS)

_CACHE = {}


def _build_graph():
    from contextlib import ExitStack
    import concourse.bass as bass
    import concourse.mybir as mybir

    dt = mybir.dt
    F32, BF16 = dt.float32, dt.bfloat16
    AF = mybir.ActivationFunctionType
    OP = mybir.AluOpType
    AX = mybir.AxisListType
    nc = bass.Bass()

    d_in = {}
    for name, shape, ty in [
        ("xq", [NQ, DIM], F32), ("xtq", [DIM, NQ], F32), ("xt", [DIM, N], BF16),
        ("wq", [DIM, INNER], BF16), ("srow", [1, INNER], BF16),
        ("wkv", [DIM, 2 * DIM_HEAD], BF16), ("nkvt", [128, NUM_NULL], BF16),
        ("wout", [INNER, DIM], BF16), ("ibf", [128, 128], BF16),
        ("if32", [128, 128], F32), ("onesbf", [1, 128], BF16),
        ("onesf", [1, 128], F32), ("biasp", [NPAIR, 128, EW], BF16),
    ]:
        d_in[name] = nc.declare_dram_parameter(name, shape, ty, isOutput=False)
    out_d = nc.declare_dram_parameter("out", [DIM, NQ], F32, isOutput=True)

    ctx = ExitStack()
    sb = {}
    for name, shape, ty in [
        ("ibf", [128, 128], BF16), ("if32", [128, 128], F32),
        ("onesbf", [1, 128], BF16), ("onesf", [1, 128], F32),
        ("srow", [1, INNER], BF16), ("zb", [128, 1], F32), ("epsb", [128, 1], F32),
        ("wq", [128, 8 * INNER], BF16), ("wout", [128, 8 * DIM], BF16),
        ("wkv", [128, 8 * 128], BF16), ("xt", [128, 8 * N], BF16),
        ("xtq", [128, 8 * NQ], F32),
        ("xrow", [128, 2 * DIM], F32), ("xc", [128, DIM], F32),
        ("lns", [128, 12], F32),
        ("rsq_row", [1, NQ], F32), ("nmr_row", [1, NQ], F32),
        ("rsqb", [128, NQ], F32), ("negmurs", [1, NQ], BF16),
        ("xst", [128, 8 * NQ], BF16), ("qtmp", [128, 2 * NQ], BF16),
        ("kv", [128, JPAD], BF16), ("vsb", [128, JT * 65], BF16),
        ("e0", [128, EW], BF16), ("e1", [128, EW], BF16),
        ("b0", [128, EW], BF16), ("b1", [128, EW], BF16),
        ("eraw0", [128, CH_LEN[0]], BF16), ("eraw1", [128, CH_LEN[1]], BF16),
        ("eraw2", [128, CH_LEN[2]], BF16),
        ("oT", [64, HEADS * NQ], BF16),
        ("rrow", [1, 512], F32), ("recipflat", [1, HEADS * NQ], BF16),
        ("oTn", [128, 8 * NQ], BF16), ("oTn_lo", [64, 8 * NQ], BF16),
        ("outsb", [128, 2 * NQ], F32),
    ] + [(f"qh{h}", [64, 2 * NQ], BF16) for h in range(NPAIR)]:
        sb[name] = ctx.enter_context(nc.sbuf_tensor("sb_" + name, shape, ty))

    qh = [sb[f"qh{h}"] for h in range(NPAIR)]
    esb = [sb["e0"], sb["e1"]]
    bsb = [sb["b0"], sb["b1"]]
    eraw = [sb["eraw0"], sb["eraw1"], sb["eraw2"]]

    # PSUM: early tensors freed before pair-loop tensors are allocated.
    early = ExitStack()
    kvp = [early.enter_context(nc.psum_tensor(f"kvp{i}", [128, 512], F32))
           for i in range(2)]
    qp = [early.enter_context(nc.psum_tensor(f"qp{i}", [128, NQ], F32))
          for i in range(2)]
    vp = [early.enter_context(nc.psum_tensor(f"vp{i}", [128, 64], BF16))
          for i in range(2)]
    stp = early.enter_context(nc.psum_tensor("stp", [1, 128], F32))
    rbp = early.enter_context(nc.psum_tensor("rbp", [128, NQ], F32))
    early.close()
    simA = ctx.enter_context(nc.psum_tensor("simA", [128, 1536], F32))
    simB = ctx.enter_context(nc.psum_tensor("simB", [128, 1536], F32))
    opp = ctx.enter_context(nc.psum_tensor("opp", [65, 512], F32))
    nrm = ctx.enter_context(nc.psum_tensor("nrm", [64, 512], F32))
    REG = {0: simA, 1: simB, 2: simA}

    # ------- planner -------
    plan = {"sync": [], "tensor": [], "vector": [], "scalar": []}
    DSEMS = (("dkw", "dx", "dxtq", "dwq", "dsm", "dw", "db0", "db1", "dn", "do")
             + tuple(f"dk{i}" for i in range(4))
             + tuple(f"dq{i}" for i in range(8)))
    cnt = {"p": 0, "v": 0, "s": 0, **{k: 0 for k in DSEMS}}
    SEM = {}

    def wait(eng, sem, thr):
        if thr > 0:
            plan[eng].append(lambda e, s=sem, t=thr: e.wait_ge(SEM[s], t))

    def dma(sem, out, in_):
        cnt[sem] += 16
        plan["sync"].append(
            lambda e, s=sem, o=out, i=in_: e.dma_start(out=o, in_=i)
            .then_inc(SEM[s], 16))
        return cnt[sem]

    def inc(eng, sem, fn):
        cnt[sem] += 1
        if eng in ("vector", "scalar"):
            plan[eng].append(lambda e, f=fn: f(e))
            plan[eng].append(lambda e, s=sem: e.drain().then_inc(SEM[s], 1))
        else:
            plan[eng].append(lambda e, f=fn, s=sem: f(e).then_inc(SEM[s], 1))
        return cnt[sem]

    def run(eng, fn):
        plan[eng].append(fn)
        if eng in ("vector", "scalar"):
            plan[eng].append(lambda e: e.drain())

    # ========== SYNC: initial loads in priority order ==========
    def dma8(sem, nm):
        return dma(sem, sb[nm][:].rearrange("p (c f) -> p c f", c=8),
                   d_in[nm][:].rearrange("(c p) f -> p c f", c=8))

    for nm in ("ibf", "if32", "onesbf", "onesf", "srow"):
        dma("dsm", sb[nm][:], d_in[nm][:])
    d_sm0 = dma("dsm", sb["kv"][:, 0:NUM_NULL], d_in["nkvt"][:])
    d_xq = dma("dx", sb["xrow"][:].rearrange("p (t f) -> p t f", t=2),
               d_in["xq"][:].rearrange("(t p) f -> p t f", t=2))
    d_xtq = dma8("dxtq", "xtq")
    d_kw = dma8("dkw", "wkv")
    # xt split into 4 column slices so kv chunk ch can start on slice ch
    for chs in range(4):
        dma(f"dk{chs}",
            sb["xt"][:].rearrange("p (c f) -> p c f", c=8)[:, :, chs * 512:(chs + 1) * 512],
            d_in["xt"][:].rearrange("(c p) f -> p c f", c=8)[:, :, chs * 512:(chs + 1) * 512])
    d_wq = dma8("dwq", "wq")
    for p in range(2):
        dma(f"db{p}", bsb[p][:], d_in["biasp"][p])
    d_w = dma8("dw", "wout")

    # ========== VECTOR: memsets ==========
    run("vector", lambda e: e.memset(sb["zb"][:], 0.0))
    run("vector", lambda e: e.memset(sb["epsb"][:], EPS))
    run("vector", lambda e: e.memset(sb["vsb"][:], 1.0))
    v_memset = inc("vector", "v",
                   lambda e: e.memset(sb["kv"][:, NUM_NULL + N:JPAD], 0.0))

    # ========== LN stats: lns cols t*6 + {0 negmu, 1 ssq, 2 lnv, 3 rsqc, 4 nmrc}
    v_center = [0, 0]
    s_sq = [0, 0]
    s_rsqc = [0, 0]
    v_nmrc = [0, 0]
    for t in range(2):
        c0 = t * 6
        negmu = sb["lns"][:, c0:c0 + 1]
        if t == 0:
            wait("vector", "dx", d_xq)
        if t == 1:
            wait("vector", "s", s_sq[0])  # xc reuse
        run("vector", lambda e, t=t, negmu=negmu: e.tensor_reduce(
            out=negmu, in_=sb["xrow"][:, t * DIM:(t + 1) * DIM],
            axis=AX.X, op=OP.add, negate=True))
        run("vector", lambda e, negmu=negmu: e.tensor_scalar_mul(
            out=negmu, in0=negmu, scalar1=1.0 / DIM))
        v_center[t] = inc("vector", "v", lambda e, t=t, negmu=negmu:
                          e.tensor_scalar_add(
                              out=sb["xc"][:],
                              in0=sb["xrow"][:, t * DIM:(t + 1) * DIM],
                              scalar1=negmu))
        # scalar chain for this t
        if t == 0:
            wait("scalar", "v", v_memset)
        wait("scalar", "v", v_center[t])
        ssq = sb["lns"][:, c0 + 1:c0 + 2]
        lnv = sb["lns"][:, c0 + 2:c0 + 3]
        rsqc = sb["lns"][:, c0 + 3:c0 + 4]
        s_sq[t] = inc("scalar", "s", lambda e, t=t, ssq=ssq: e.activation(
            out=sb["xrow"][:, t * DIM:(t + 1) * DIM], in_=sb["xc"][:],
            func=AF.Square, bias=sb["zb"][:], accum_out=ssq))
        run("scalar", lambda e, ssq=ssq, lnv=lnv: e.activation(
            out=lnv, in_=ssq, func=AF.Ln, scale=1.0 / DIM, bias=sb["epsb"][:]))
        s_rsqc[t] = inc("scalar", "s", lambda e, lnv=lnv, rsqc=rsqc: e.activation(
            out=rsqc, in_=lnv, func=AF.Exp, scale=-0.5, bias=sb["zb"][:]))
        wait("vector", "s", s_rsqc[t])
        v_nmrc[t] = inc("vector", "v", lambda e, c0=c0: e.tensor_tensor(
            out=sb["lns"][:, c0 + 4:c0 + 5], in0=sb["lns"][:, c0:c0 + 1],
            in1=sb["lns"][:, c0 + 3:c0 + 4], op=OP.mult))

    # ========== TENSOR: kv matmuls (kvp double-buffered) ==========
    p_kvchunk = [0] * 4
    s_kvevac = [0] * 4
    for ch in range(4):
        pb = kvp[ch % 2]
        if ch == 0:
            wait("tensor", "dkw", d_kw)
        wait("tensor", f"dk{ch}", 16)
        if ch >= 2:
            wait("tensor", "s", s_kvevac[ch - 2])
        for ct in range(8):
            fn = lambda e, pb=pb, ch=ch, ct=ct: e.matmul(
                pb[:], sb["wkv"][:, ct * 128:(ct + 1) * 128],
                sb["xt"][:, ct * N + ch * 512:ct * N + (ch + 1) * 512],
                start=(ct == 0), stop=(ct == 7))
            if ct == 7:
                p_kvchunk[ch] = inc("tensor", "p", fn)
            else:
                run("tensor", fn)
        wait("scalar", "p", p_kvchunk[ch])
        s_kvevac[ch] = inc("scalar", "s", lambda e, pb=pb, ch=ch: e.activation(
            out=sb["kv"][:, NUM_NULL + ch * 512:NUM_NULL + (ch + 1) * 512],
            in_=pb[:], func=AF.Copy))

    # ========== TENSOR: stats transposes + rsqb broadcast ==========
    v_statrow = [[0, 0], [0, 0]]
    wait("tensor", "dsm", d_sm0)  # if32/onesf/ibf/srow/nkvt loaded
    for t in range(2):
        c0 = t * 6
        wait("tensor", "s", s_rsqc[t])
        if t == 1:
            wait("tensor", "v", v_statrow[0][1])  # stp reuse
        pst = inc("tensor", "p", lambda e, c0=c0: e.transpose(
            stp[:], sb["lns"][:, c0 + 3:c0 + 4], sb["if32"][:]))
        wait("vector", "p", pst)
        v_statrow[t][0] = inc("vector", "v", lambda e, t=t: e.tensor_copy(
            sb["rsq_row"][0:1, t * 128:(t + 1) * 128], stp[:]))
        wait("tensor", "v", v_statrow[t][0])
        wait("tensor", "v", v_nmrc[t])
        pst2 = inc("tensor", "p", lambda e, c0=c0: e.transpose(
            stp[:], sb["lns"][:, c0 + 4:c0 + 5], sb["if32"][:]))
        wait("vector", "p", pst2)
        v_statrow[t][1] = inc("vector", "v", lambda e, t=t: e.tensor_copy(
            sb["nmr_row"][0:1, t * 128:(t + 1) * 128], stp[:]))

    wait("tensor", "v", v_statrow[1][0])
    p_rsqb = inc("tensor", "p", lambda e: e.matmul(
        rbp[:], sb["onesf"][0:1, :], sb["rsq_row"][0:1, :], start=True, stop=True))
    wait("vector", "p", p_rsqb)
    wait("vector", "dxtq", d_xtq)
    run("vector", lambda e: e.tensor_copy(sb["rsqb"][:], rbp[:]))
    v_negmurs = inc("vector", "v",
                    lambda e: e.tensor_copy(sb["negmurs"][:], sb["nmr_row"][0:1, :]))
    for ct in range(8):
        fn = lambda e, ct=ct: e.tensor_tensor(
            out=sb["xst"][:, ct * NQ:(ct + 1) * NQ],
            in0=sb["xtq"][:, ct * NQ:(ct + 1) * NQ], in1=sb["rsqb"][:], op=OP.mult)
        if ct == 7:
            v_xst = inc("vector", "v", fn)
        else:
            run("vector", fn)

    # ========== TENSOR: v transposes (vp double-buffered) ==========
    p_vt = [0] * JT
    v_vcopy = [0] * JT
    for jt in range(JT):
        pb = vp[jt % 2]
        ch_hi = min(3, ((jt + 1) * 128 - 1 - NUM_NULL) // 512)
        wait("tensor", "s", s_kvevac[ch_hi])
        if jt == JT - 1:
            wait("tensor", "v", v_memset)
        if jt >= 2:
            wait("tensor", "v", v_vcopy[jt - 2])
        p_vt[jt] = inc("tensor", "p", lambda e, pb=pb, jt=jt: e.transpose(
            pb[:], sb["kv"][64:128, jt * 128:(jt + 1) * 128],
            sb["ibf"][64:128, 64:128]))
        wait("vector", "p", p_vt[jt])
        v_vcopy[jt] = inc("vector", "v", lambda e, pb=pb, jt=jt: e.tensor_copy(
            sb["vsb"][:, jt * 65:jt * 65 + 64], pb[:]))
    v_vsb = v_vcopy[JT - 1]

    # ========== TENSOR: q projection (qp double-buffered), head-major evac ===
    wait("tensor", "v", v_xst)
    wait("tensor", "dwq", d_wq)
    p_q = [0] * 8
    v_qtmp = [0] * 8
    for dtile in range(8):
        pb = qp[dtile % 2]
        if dtile >= 2:
            wait("tensor", "v", v_qtmp[dtile - 2])
        for ct in range(8):
            run("tensor", lambda e, pb=pb, dtile=dtile, ct=ct: e.matmul(
                pb[:],
                sb["wq"][:, ct * INNER + dtile * 128:ct * INNER + (dtile + 1) * 128],
                sb["xst"][:, ct * NQ:(ct + 1) * NQ],
                start=(ct == 0), stop=False))
        p_q[dtile] = inc("tensor", "p", lambda e, pb=pb, dtile=dtile: e.matmul(
            pb[:], sb["srow"][0:1, dtile * 128:(dtile + 1) * 128],
            sb["negmurs"][:], start=False, stop=True))
        wait("vector", "p", p_q[dtile])
        # even head (psum rows 0:64) -> qh[p][:, 0:256] directly
        run("vector", lambda e, pb=pb, dtile=dtile: e.tensor_copy(
            qh[dtile][0:64, 0:NQ], pb[0:64, :]))
        slot = dtile % 2
        if dtile >= 2:
            wait("vector", f"dq{dtile - 2}", 16)  # qtmp slot reuse
        v_qtmp[dtile] = inc("vector", "v", lambda e, pb=pb, slot=slot:
                            e.tensor_copy(
                                sb["qtmp"][64:128, slot * NQ:(slot + 1) * NQ],
                                pb[64:128, :]))
        wait("sync", "v", v_qtmp[dtile])
        dma(f"dq{dtile}", qh[dtile][0:64, NQ:2 * NQ],
            sb["qtmp"][64:128, slot * NQ:(slot + 1) * NQ])

    # ========== PAIR LOOP ==========
    v_pre = max(v_vsb, v_qtmp[7])
    p_simc = [[0] * NCK for _ in range(NPAIR)]
    s_exp = [[0] * NCK for _ in range(NPAIR)]
    v_mult = [[0] * NCK for _ in range(NPAIR)]
    p_odone = [0] * NPAIR
    v_oevac = [0] * NPAIR
    p_bcast = [0] * NPAIR
    v_normmult = [0] * NPAIR

    def emit_fill(p, ci):
        ps = REG[ci]
        eh_ = esb[p % 2]
        # psum region reuse: previous occupant is (p-1, 2) for A/ck0,
        # (p-1, 1) for B/ck1, (p, 0) for A/ck2
        if ci == 0 and p >= 1:
            wait("tensor", "s", s_exp[p - 1][2])
        if ci == 1 and p >= 1:
            wait("tensor", "s", s_exp[p - 1][1])
        if ci == 2:
            wait("tensor", "s", s_exp[p][0])
        if p == 0 and ci == 0:
            wait("tensor", "v", v_pre)      # early psum drained
        if ci == 0:
            wait("tensor", f"dq{p}", 16)
        ch_hi = min(3, ((CHUNKS[ci][-1] + 1) * 128 - 1 - NUM_NULL) // 512)
        if p == 0:
            wait("tensor", "s", s_kvevac[ch_hi])
        base = CH_E1[ci]
        half = CH_LEN[ci] // 2
        # build emission list of bank-safe pieces, then set start on the
        # first piece touching each psum bank and stop on the last (start
        # zeroes the whole 2KB bank; one start/stop pair per bank per group)
        pieces = []
        for jt in CHUNKS[ci]:
            s, w = S_JT[jt], W_JT[jt]
            for h in range(2):
                a0 = h * half + (ECOL1[jt] - base)
                a, b = a0, a0 + w
                while a < b:
                    cut = min(b, (a // 512 + 1) * 512)
                    qa = h * 256 + s + (a - a0)
                    pieces.append([jt, a, cut, qa, qa + (cut - a)])
                    a = cut
        first_in_bank = {}
        last_in_bank = {}
        for pi, (jt, a, b, qa, qb) in enumerate(pieces):
            first_in_bank.setdefault(a // 512, pi)
            last_in_bank[a // 512] = pi
        for pi, (jt, a, b, qa, qb) in enumerate(pieces):
            st = first_in_bank[a // 512] == pi
            sp = last_in_bank[a // 512] == pi
            fn = lambda e, ps=ps, jt=jt, a=a, b=b, qa=qa, qb=qb, st=st, \
                sp=sp: e.matmul(
                    ps[:, a:b], sb["kv"][0:64, jt * 128:(jt + 1) * 128],
                    qh[p][0:64, qa:qb], start=st, stop=sp)
            if pi == len(pieces) - 1:
                p_simc[p][ci] = inc("tensor", "p", fn)
            else:
                run("tensor", fn)

    def emit_ogroup(p):
        eh_ = esb[p % 2]
        if p == 0:
            wait("tensor", "v", v_vsb)
        if p >= 1:
            wait("tensor", "v", v_oevac[p - 1])   # opp reuse
        for ci in range(NCK):
            wait("tensor", "v", v_mult[p][ci])
            for jt in CHUNKS[ci]:
                s, w = S_JT[jt], W_JT[jt]
                for h in range(2):
                    fn = lambda e, jt=jt, s=s, w=w, h=h, eh_=eh_: e.matmul(
                        opp[0:65, h * 256 + s:(h + 1) * 256],
                        sb["vsb"][:, jt * 65:jt * 65 + 65],
                        eh_[:, h * EW1 + ECOL1[jt]:h * EW1 + ECOL1[jt] + w],
                        start=(jt == 0 and h == 0),
                        stop=(jt == JT - 1 and h == 1))
                    if jt == JT - 1 and h == 1:
                        p_odone[p] = inc("tensor", "p", fn)
                    else:
                        run("tensor", fn)

    def emit_bcast(p):
        wait("tensor", "v", v_oevac[p])      # recipflat(p) ready
        if p >= 1:
            wait("tensor", "v", v_normmult[p - 1])   # nrm reuse
        p_bcast[p] = inc("tensor", "p", lambda e, p=p: e.matmul(
            nrm[:], sb["onesbf"][0:1, 0:64],
            sb["recipflat"][0:1, p * 512:(p + 1) * 512], start=True, stop=True))

    def emit_mults(p, ci):
        eh_ = esb[p % 2]
        bh_ = bsb[p % 2]
        wait("vector", "s", s_exp[p][ci])
        if ci == 0:
            wait("vector", f"db{p % 2}", 16 * (p // 2 + 1))
            if p >= 2:
                wait("vector", "p", p_odone[p - 2])   # eh slot reuse
        base = CH_E1[ci]
        half = CH_LEN[ci] // 2
        run("vector", lambda e, half=half, base=base, eh_=eh_, bh_=bh_,
            ci=ci: e.tensor_tensor(
                out=eh_[:, base:base + half], in0=eraw[ci][:, 0:half],
                in1=bh_[:, base:base + half], op=OP.mult))
        v_mult[p][ci] = inc("vector", "v", lambda e, half=half, base=base,
                            eh_=eh_, bh_=bh_, ci=ci: e.tensor_tensor(
            out=eh_[:, EW1 + base:EW1 + base + half],
            in0=eraw[ci][:, half:2 * half],
            in1=bh_[:, EW1 + base:EW1 + base + half], op=OP.mult))

    def emit_oevac(p):
        wait("vector", "p", p_odone[p])
        run("vector", lambda e, p=p: e.tensor_copy(
            sb["oT"][0:64, p * 512:(p + 1) * 512], opp[0:64, :]))
        run("vector", lambda e: e.reciprocal(
            out=sb["rrow"][0:1, :], in_=opp[64:65, :]))
        v_oevac[p] = inc("vector", "v", lambda e, p=p: e.tensor_copy(
            sb["recipflat"][0:1, p * 512:(p + 1) * 512], sb["rrow"][0:1, :]))

    def emit_normmult(p):
        wait("vector", "p", p_bcast[p])
        run("vector", lambda e, p=p: e.tensor_tensor(
            out=sb["oTn"][0:64, p * NQ:(p + 1) * NQ], in0=nrm[0:64, 0:256],
            in1=sb["oT"][0:64, p * 512:p * 512 + 256], op=OP.mult))
        v_normmult[p] = inc("vector", "v", lambda e, p=p: e.tensor_tensor(
            out=sb["oTn_lo"][0:64, p * NQ:(p + 1) * NQ], in0=nrm[0:64, 256:512],
            in1=sb["oT"][0:64, p * 512 + 256:(p + 1) * 512], op=OP.mult))

    def emit_exp(p, ci):
        wait("scalar", "p", p_simc[p][ci])
        if p >= 1:
            wait("scalar", "v", v_mult[p - 1][ci])    # eraw slot reuse
        ps = REG[ci]
        ln = CH_LEN[ci]
        s_exp[p][ci] = inc("scalar", "s", lambda e, ps=ps, ln=ln, ci=ci:
                           e.activation(out=eraw[ci][:, 0:ln], in_=ps[:, 0:ln],
                                        func=AF.Exp, bias=sb["zb"][:]))

    for p in range(NPAIR):
        # call order matters only for plan-time counter availability;
        # engine programs are built per-engine in the order emitted below
        emit_fill(p, 0)
        emit_fill(p, 1)
        emit_exp(p, 0)
        emit_exp(p, 1)
        emit_mults(p, 0)
        if p >= 1:
            emit_ogroup(p - 1)
        if p >= 2:
            emit_bcast(p - 2)
        emit_fill(p, 2)
        emit_exp(p, 2)
        emit_mults(p, 1)
        if p >= 1:
            emit_oevac(p - 1)
        if p >= 2:
            emit_normmult(p - 2)
        emit_mults(p, 2)
        # SYNC: bias prefetch for pair p+2; oTn odd-half shuffle for p-2
        if p + 2 < NPAIR:
            wait("sync", "v", v_mult[p][2])
            dma(f"db{(p + 2) % 2}", bsb[(p + 2) % 2][:], d_in["biasp"][p + 2])
        if p >= 2:
            wait("sync", "v", v_normmult[p - 2])
            dma("dn", sb["oTn"][64:128, (p - 2) * NQ:(p - 1) * NQ],
                sb["oTn_lo"][0:64, (p - 2) * NQ:(p - 1) * NQ])

    # epilogue
    emit_ogroup(NPAIR - 1)
    emit_bcast(NPAIR - 2)
    emit_oevac(NPAIR - 1)
    emit_normmult(NPAIR - 2)
    emit_bcast(NPAIR - 1)
    emit_normmult(NPAIR - 1)
    for p in (NPAIR - 2, NPAIR - 1):
        wait("sync", "v", v_normmult[p])
        dma("dn", sb["oTn"][64:128, p * NQ:(p + 1) * NQ],
            sb["oTn_lo"][0:64, p * NQ:(p + 1) * NQ])

    # ========== output projection (accumulators alternate in simA) ==========
    wait("tensor", "v", v_normmult[NPAIR - 1])
    wait("tensor", "dn", 16 * 8)
    wait("tensor", "dw", d_w)
    wait("tensor", "s", s_exp[NPAIR - 1][2])   # simA free
    s_outevac = [0] * 8
    for et in range(8):
        reg = simA[:, 0:NQ] if et % 2 == 0 else simA[:, 512:512 + NQ]
        if et >= 2:
            wait("tensor", "s", s_outevac[et - 2])
        p_wout = 0
        for hdt in range(8):
            fn = lambda e, et=et, hdt=hdt, reg=reg: e.matmul(
                reg, sb["wout"][:, hdt * DIM + et * 128:hdt * DIM + (et + 1) * 128],
                sb["oTn"][:, hdt * NQ:(hdt + 1) * NQ],
                start=(hdt == 0), stop=(hdt == 7))
            if hdt == 7:
                p_wout = inc("tensor", "p", fn)
            else:
                run("tensor", fn)
        wait("scalar", "p", p_wout)
        oslot = et % 2
        if et >= 2:
            wait("scalar", "do", 16 * et)  # outsb slot reuse (all issued)
        s_outevac[et] = inc("scalar", "s", lambda e, oslot=oslot, reg=reg:
                            e.activation(
                                out=sb["outsb"][:, oslot * NQ:(oslot + 1) * NQ],
                                in_=reg, func=AF.Copy))
        wait("sync", "s", s_outevac[et])
        dma("do", out_d[et * 128:(et + 1) * 128, :],
            sb["outsb"][:, oslot * NQ:(oslot + 1) * NQ])

    # ========== emit ==========
    from contextlib import ExitStack as _ES
    semctx = _ES()
    for k in ("p", "v", "s") + DSEMS:
        SEM[k] = semctx.enter_context(nc.semaphore(f"sem_{k}"))
    with semctx:
        with nc.Block() as block:
            @block.sync
            def _(e):
                for fn in plan["sync"]:
                    fn(e)

            @block.tensor
            def _(e):
                for fn in plan["tensor"]:
                    fn(e)

            @block.vector
            def _(e):
                for fn in plan["vector"]:
                    fn(e)

            @block.scalar
            def _(e):
                for fn in plan["scalar"]:
                    fn(e)
    ctx.close()
    return nc


def _prep_inputs(x, attn_bias, Wq, Wkv, null_kv, Wout, gamma, mask):
    from ml_dtypes import bfloat16
    x = np.asarray(x, np.float32)[0]            # [N, DIM]
    attn_bias = np.asarray(attn_bias, np.float32)[0]  # [H, N, N]
    Wq = np.asarray(Wq, np.float32)
    Wkv = np.asarray(Wkv, np.float32)
    null_kv = np.asarray(null_kv, np.float32)
    Wout = np.asarray(Wout, np.float32)
    gamma = np.asarray(gamma, np.float32)
    mask = np.asarray(mask, bool)[0]            # [N]

    scale = DIM_HEAD ** -0.5
    wq_eff = (gamma[:, None] * Wq * scale).astype(np.float32)
    srow = wq_eff.sum(axis=0, keepdims=True)
    xt = np.ascontiguousarray(x.T)
    nkvt = np.zeros((128, NUM_NULL), np.float32)
    nkvt[0:DIM_HEAD, :] = null_kv[0].T
    nkvt[64:64 + DIM_HEAD, :] = null_kv[1].T
    I128 = np.eye(128, dtype=np.float32)
    ones = np.ones((1, 128), np.float32)

    jpad = np.arange(JPAD)
    jvalid = np.zeros(JPAD, bool)
    jvalid[:NUM_NULL] = True
    jvalid[NUM_NULL:NUM_NULL + N] = mask
    key_of_j = jpad - NUM_NULL

    in_maps = []
    idx_all = []
    for c in range(NCORES):
        idx = np.concatenate([np.arange(c, 1024, 8), np.arange(1024 + c, 2048, 8)])
        idx_all.append(idx)
        allow = jvalid[None, :] & (key_of_j[None, :] <= idx[:, None])  # [NQ, JPAD]
        allow[:, :NUM_NULL] = True
        ab = np.zeros((HEADS, JPAD, NQ), np.float32)
        ab[:, NUM_NULL:NUM_NULL + N, :] = attn_bias[:, idx, :].transpose(0, 2, 1)
        bt = np.where(allow.T[None], ab, MASK_VAL)
        ebt = np.exp(bt)                 # exp(bias); exactly 0 where masked
        # pack per head-PAIR, trimmed per-tile: [h0 (w) | h1 (w)] at ECOL[jt]
        pk = np.empty((NPAIR, 128, EW), np.float32)
        for jt in range(JT):
            s, w = S_JT[jt], W_JT[jt]
            c0 = ECOL1[jt]
            tile = ebt[:, jt * 128:(jt + 1) * 128, s:256]     # [H, 128, w]
            pk[:, :, c0:c0 + w] = tile[0::2]
            pk[:, :, EW1 + c0:EW1 + c0 + w] = tile[1::2]
        in_maps.append({
            "xq": np.ascontiguousarray(x[idx]),
            "xtq": np.ascontiguousarray(xt[:, idx]),
            "xt": xt.astype(bfloat16),
            "wq": wq_eff.astype(bfloat16),
            "srow": srow.astype(bfloat16),
            "wkv": Wkv.astype(bfloat16),
            "nkvt": nkvt.astype(bfloat16),
            "wout": Wout.astype(bfloat16),
            "ibf": I128.astype(bfloat16),
            "if32": I128,
            "onesbf": ones.astype(bfloat16),
            "onesf": ones,
            "biasp": pk.astype(bfloat16),
        })
    return in_maps, idx_all


def _run(inputs, trace=False):
    from concourse.bass_utils import run_bass_kernel_spmd
    if "nc" not in _CACHE:
        _CACHE["nc"] = _build_graph()
    nc = _CACHE["nc"]
    in_maps, idx_all = _prep_inputs(**inputs)
    res = run_bass_kernel_spmd(nc, in_maps, list(range(NCORES)), trace=trace)
    out = np.zeros((B, N, DIM), np.float32)
    for c in range(NCORES):
        out[0, idx_all[c], :] = res.results[c]["out"].T
    return out, res


def kernel(**inputs):
    out, _ = _run(inputs, trace=False)
    return out


# revision 12
# speedup vs baseline: 1.6838x; 1.0880x over previous
"""Trainium2 8-core kernel for nn_Attention_70892730187933 (sparse multi-query attention).

Sharding: sequence-parallel over query rows. Core c owns rows {i : i % 8 == c},
as 2 blocks of 128 rows (block0 < 1024, block1 >= 1024). Key space padded to
17*128 = 2176 (incl. 2 null cols). No collectives; host concatenates rows.

v2 design vs baseline:
- Causal q-column trimming: for key tile jt, only q columns [S_jt, 256) per
  head can attend (S_jt = max(0, 16*jt-1), worst-case over cores); per-pair
  sim width drops 6656 -> 4384. q is packed head-major so each tile is one
  contiguous per-head slice.
- Bias is applied multiplicatively: host packs exp(bias) (0 where masked) and
  DVE multiplies it into exp(sim) at bf16 2x rate. This removes the
  identity-matmul bias adds (half of all sim PE work) entirely.
- Softmax normalization is inline: rowsums ride as a ones-column in V; DVE
  takes reciprocals straight from the PSUM rowsum row (no DRAM roundtrips)
  and per-head normalization overlaps the pair loop.
- DMA issue order prioritizes the critical path: consts, xq/xtq (LN chain),
  wkv + xt (kv chain, split in 4 column slices gating kv matmul chunks), wq,
  then bias tiles / wout.

Raw Block + explicit semaphores (this walrus build rejects multi-wait
instructions); the planner records semaphore counter targets at plan time,
then emits all four engine programs inside one Block.
"""

import sys
import numpy as np

sys.path.insert(0, "/opt/trn_rl_repo")

B, N, DIM, HEADS, DIM_HEAD, NUM_NULL = 1, 2048, 1024, 16, 64, 2
INNER = HEADS * DIM_HEAD
EPS = 1e-5
NCORES = 8
JT = 17
JPAD = JT * 128
NQ = 256
MASK_VAL = -30000.0
NPAIR = HEADS // 2

# per-head q-col start for key tile jt (worst case over cores => widest)
S_JT = [max(0, 16 * jt - 1) for jt in range(JT)]
W_JT = [256 - s for s in S_JT]          # per-head width
TW = [2 * w for w in W_JT]              # per-pair tile width (h0|h1 packed)
# E/bias pair layout: [h0 tiles packed (EW1) | h1 tiles packed (EW1)]
ECOL1 = [0] * JT
for _jt in range(1, JT):
    ECOL1[_jt] = ECOL1[_jt - 1] + W_JT[_jt - 1]
EW1 = ECOL1[-1] + W_JT[-1]              # 2192 per-head packed width
EW = 2 * EW1                            # 4384 packed pair E/bias width

# sim psum chunks: tiles packed into regions A(<=1536), B(<=1536), A(<=1536)
# region layout per chunk: [h0 tiles (L/2) | h1 tiles (L/2)]
CHUNKS = [[0, 1, 2], [3, 4, 5, 6], list(range(7, JT))]
CH_LEN = [sum(TW[j] for j in ck) for ck in CHUNKS]    # 1444, 1480, 1460
CH_E1 = [ECOL1[ck[0]] for ck in CHUNKS]               # per-head col offset
NCK = len(CHUNKS)

_CACHE = {}


def _build_graph():
    from contextlib import ExitStack
    import concourse.bass as bass
    import concourse.mybir as mybir

    dt = mybir.dt
    F32, BF16 = dt.float32, dt.bfloat16
    AF = mybir.ActivationFunctionType
    OP = mybir.AluOpType
    AX = mybir.AxisListType
    nc = bass.Bass()

    # all DRAM parameters are host-prearranged to match their SBUF layout
    # exactly (contiguous [128, W] rows -> 128 large DMA descriptors each);
    # xt is chunk-slice-major: [p, chs, ct, 512] so each of the 4 column
    # slices is one contiguous row-chunk
    d_in = {}
    for name, shape, ty in [
        ("xq", [128, 2 * DIM], F32), ("xtq", [128, 8 * NQ], F32),
        ("xt", [128, 8 * N], BF16),
        ("wq", [128, 8 * INNER], BF16), ("srow", [1, INNER], BF16),
        ("wkv", [128, 8 * 128], BF16), ("nkvt", [128, NUM_NULL], BF16),
        ("wout", [128, 8 * DIM], BF16), ("ibf", [128, 128], BF16),
        ("if32", [128, 128], F32), ("onesbf", [1, 128], BF16),
        ("onesf", [1, 128], F32), ("biasp", [NPAIR, 128, EW], BF16),
    ]:
        d_in[name] = nc.declare_dram_parameter(name, shape, ty, isOutput=False)
    out_d = nc.declare_dram_parameter("out", [DIM, NQ], F32, isOutput=True)

    ctx = ExitStack()
    sb = {}
    for name, shape, ty in [
        ("ibf", [128, 128], BF16), ("if32", [128, 128], F32),
        ("onesbf", [1, 128], BF16), ("onesf", [1, 128], F32),
        ("srow", [1, INNER], BF16), ("zb", [128, 1], F32), ("epsb", [128, 1], F32),
        ("wq", [128, 8 * INNER], BF16), ("wout", [128, 8 * DIM], BF16),
        ("wkv", [128, 8 * 128], BF16), ("xt", [128, 8 * N], BF16),
        ("xtq", [128, 8 * NQ], F32),
        ("xrow", [128, 2 * DIM], F32), ("xc", [128, DIM], F32),
        ("lns", [128, 12], F32),
        ("rsq_row", [1, NQ], F32), ("nmr_row", [1, NQ], F32),
        ("rsqb", [128, NQ], F32), ("negmurs", [1, NQ], BF16),
        ("xst", [128, 8 * NQ], BF16), ("qtmp", [128, 2 * NQ], BF16),
        ("kv", [128, JPAD], BF16), ("vsb", [128, JT * 65], BF16),
        ("e0", [128, EW], BF16), ("e1", [128, EW], BF16),
        ("b0", [128, EW], BF16), ("b1", [128, EW], BF16),
        ("eraw0", [128, CH_LEN[0]], BF16), ("eraw1", [128, CH_LEN[1]], BF16),
        ("eraw2", [128, CH_LEN[2]], BF16),
        ("oT", [64, HEADS * NQ], BF16),
        ("rrow", [1, 512], F32), ("recipflat", [1, HEADS * NQ], BF16),
        ("oTn", [128, 8 * NQ], BF16), ("oTn_lo", [64, 8 * NQ], BF16),
        ("outsb", [128, 3 * NQ], F32),
    ] + [(f"qh{h}", [64, 2 * NQ], BF16) for h in range(NPAIR)]:
        sb[name] = ctx.enter_context(nc.sbuf_tensor("sb_" + name, shape, ty))

    qh = [sb[f"qh{h}"] for h in range(NPAIR)]
    esb = [sb["e0"], sb["e1"]]
    bsb = [sb["b0"], sb["b1"]]
    eraw = [sb["eraw0"], sb["eraw1"], sb["eraw2"]]

    # PSUM: early tensors freed before pair-loop tensors are allocated.
    early = ExitStack()
    kvp = [early.enter_context(nc.psum_tensor(f"kvp{i}", [128, 512], F32))
           for i in range(2)]
    qp = [early.enter_context(nc.psum_tensor(f"qp{i}", [128, NQ], F32))
          for i in range(2)]
    vp = [early.enter_context(nc.psum_tensor(f"vp{i}", [128, 64], BF16))
          for i in range(2)]
    stp = early.enter_context(nc.psum_tensor("stp", [1, 128], F32))
    rbp = early.enter_context(nc.psum_tensor("rbp", [128, NQ], F32))
    early.close()
    simA = ctx.enter_context(nc.psum_tensor("simA", [128, 1536], F32))
    simB = ctx.enter_context(nc.psum_tensor("simB", [128, 1536], F32))
    opp = ctx.enter_context(nc.psum_tensor("opp", [65, 512], F32))
    nrm = ctx.enter_context(nc.psum_tensor("nrm", [64, 512], F32))
    SIMREG = [simA, simB]

    # ------- planner -------
    plan = {"sync": [], "tensor": [], "vector": [], "scalar": []}
    DSEMS = (("dkw", "dx", "dxtq", "dwq", "dsm", "dw", "db0", "db1", "dn",
              "do0", "do1", "do2")
             + tuple(f"dk{i}" for i in range(4))
             + tuple(f"dq{i}" for i in range(8)))
    cnt = {"p": 0, "v": 0, "s": 0, **{k: 0 for k in DSEMS}}
    SEM = {}

    def wait(eng, sem, thr):
        if thr > 0:
            plan[eng].append(lambda e, s=sem, t=thr: e.wait_ge(SEM[s], t))

    def dma(sem, out, in_):
        cnt[sem] += 16
        plan["sync"].append(
            lambda e, s=sem, o=out, i=in_: e.dma_start(out=o, in_=i)
            .then_inc(SEM[s], 16))
        return cnt[sem]

    def inc(eng, sem, fn):
        cnt[sem] += 1
        if eng in ("vector", "scalar"):
            plan[eng].append(lambda e, f=fn: f(e))
            plan[eng].append(lambda e, s=sem: e.drain().then_inc(SEM[s], 1))
        else:
            plan[eng].append(lambda e, f=fn, s=sem: f(e).then_inc(SEM[s], 1))
        return cnt[sem]

    def run(eng, fn):
        plan[eng].append(fn)
        if eng in ("vector", "scalar"):
            plan[eng].append(lambda e: e.drain())

    # ========== SYNC: initial loads in priority order ==========
    for nm in ("ibf", "if32", "onesbf", "onesf", "srow"):
        dma("dsm", sb[nm][:], d_in[nm][:])
    d_sm0 = dma("dsm", sb["kv"][:, 0:NUM_NULL], d_in["nkvt"][:])
    d_xq = dma("dx", sb["xrow"][:], d_in["xq"][:])
    d_xtq = dma("dxtq", sb["xtq"][:], d_in["xtq"][:])
    d_kw = dma("dkw", sb["wkv"][:], d_in["wkv"][:])
    # xt split into 4 contiguous slices so kv matmul chunk ch gates on slice ch
    for chs in range(4):
        dma(f"dk{chs}", sb["xt"][:, chs * 4096:(chs + 1) * 4096],
            d_in["xt"][:, chs * 4096:(chs + 1) * 4096])
    d_wq = dma("dwq", sb["wq"][:], d_in["wq"][:])
    for p in range(2):
        dma(f"db{p}", bsb[p][:], d_in["biasp"][p])
    d_w = dma("dw", sb["wout"][:], d_in["wout"][:])

    # ========== VECTOR: memsets ==========
    run("vector", lambda e: e.memset(sb["zb"][:], 0.0))
    run("vector", lambda e: e.memset(sb["epsb"][:], EPS))
    run("vector", lambda e: e.memset(sb["vsb"][:], 1.0))
    v_memset = inc("vector", "v",
                   lambda e: e.memset(sb["kv"][:, NUM_NULL + N:JPAD], 0.0))

    # ========== LN stats: lns cols t*6 + {0 negmu, 1 ssq, 2 lnv, 3 rsqc, 4 nmrc}
    v_center = [0, 0]
    s_sq = [0, 0]
    s_rsqc = [0, 0]
    v_nmrc = [0, 0]
    for t in range(2):
        c0 = t * 6
        negmu = sb["lns"][:, c0:c0 + 1]
        if t == 0:
            wait("vector", "dx", d_xq)
        if t == 1:
            wait("vector", "s", s_sq[0])  # xc reuse
        run("vector", lambda e, t=t, negmu=negmu: e.tensor_reduce(
            out=negmu, in_=sb["xrow"][:, t * DIM:(t + 1) * DIM],
            axis=AX.X, op=OP.add, negate=True))
        run("vector", lambda e, negmu=negmu: e.tensor_scalar_mul(
            out=negmu, in0=negmu, scalar1=1.0 / DIM))
        v_center[t] = inc("vector", "v", lambda e, t=t, negmu=negmu:
                          e.tensor_scalar_add(
                              out=sb["xc"][:],
                              in0=sb["xrow"][:, t * DIM:(t + 1) * DIM],
                              scalar1=negmu))
        # scalar chain for this t
        if t == 0:
            wait("scalar", "v", v_memset)
        wait("scalar", "v", v_center[t])
        ssq = sb["lns"][:, c0 + 1:c0 + 2]
        lnv = sb["lns"][:, c0 + 2:c0 + 3]
        rsqc = sb["lns"][:, c0 + 3:c0 + 4]
        s_sq[t] = inc("scalar", "s", lambda e, t=t, ssq=ssq: e.activation(
            out=sb["xrow"][:, t * DIM:(t + 1) * DIM], in_=sb["xc"][:],
            func=AF.Square, bias=sb["zb"][:], accum_out=ssq))
        run("scalar", lambda e, ssq=ssq, lnv=lnv: e.activation(
            out=lnv, in_=ssq, func=AF.Ln, scale=1.0 / DIM, bias=sb["epsb"][:]))
        s_rsqc[t] = inc("scalar", "s", lambda e, lnv=lnv, rsqc=rsqc: e.activation(
            out=rsqc, in_=lnv, func=AF.Exp, scale=-0.5, bias=sb["zb"][:]))
        wait("vector", "s", s_rsqc[t])
        v_nmrc[t] = inc("vector", "v", lambda e, c0=c0: e.tensor_tensor(
            out=sb["lns"][:, c0 + 4:c0 + 5], in0=sb["lns"][:, c0:c0 + 1],
            in1=sb["lns"][:, c0 + 3:c0 + 4], op=OP.mult))

    # ========== TENSOR: kv matmuls (kvp double-buffered) ==========
    p_kvchunk = [0] * 4
    s_kvevac = [0] * 4
    for ch in range(4):
        pb = kvp[ch % 2]
        if ch == 0:
            wait("tensor", "dkw", d_kw)
        wait("tensor", f"dk{ch}", 16)
        if ch >= 2:
            wait("tensor", "s", s_kvevac[ch - 2])
        for ct in range(8):
            fn = lambda e, pb=pb, ch=ch, ct=ct: e.matmul(
                pb[:], sb["wkv"][:, ct * 128:(ct + 1) * 128],
                sb["xt"][:, ch * 4096 + ct * 512:ch * 4096 + (ct + 1) * 512],
                start=(ct == 0), stop=(ct == 7))
            if ct == 7:
                p_kvchunk[ch] = inc("tensor", "p", fn)
            else:
                run("tensor", fn)
        wait("scalar", "p", p_kvchunk[ch])
        s_kvevac[ch] = inc("scalar", "s", lambda e, pb=pb, ch=ch: e.activation(
            out=sb["kv"][:, NUM_NULL + ch * 512:NUM_NULL + (ch + 1) * 512],
            in_=pb[:], func=AF.Copy))

    # ========== TENSOR: stats transposes + rsqb broadcast ==========
    v_statrow = [[0, 0], [0, 0]]
    wait("tensor", "dsm", d_sm0)  # if32/onesf/ibf/srow/nkvt loaded
    for t in range(2):
        c0 = t * 6
        wait("tensor", "s", s_rsqc[t])
        if t == 1:
            wait("tensor", "v", v_statrow[0][1])  # stp reuse
        pst = inc("tensor", "p", lambda e, c0=c0: e.transpose(
            stp[:], sb["lns"][:, c0 + 3:c0 + 4], sb["if32"][:]))
        wait("vector", "p", pst)
        v_statrow[t][0] = inc("vector", "v", lambda e, t=t: e.tensor_copy(
            sb["rsq_row"][0:1, t * 128:(t + 1) * 128], stp[:]))
        wait("tensor", "v", v_statrow[t][0])
        wait("tensor", "v", v_nmrc[t])
        pst2 = inc("tensor", "p", lambda e, c0=c0: e.transpose(
            stp[:], sb["lns"][:, c0 + 4:c0 + 5], sb["if32"][:]))
        wait("vector", "p", pst2)
        v_statrow[t][1] = inc("vector", "v", lambda e, t=t: e.tensor_copy(
            sb["nmr_row"][0:1, t * 128:(t + 1) * 128], stp[:]))

    wait("tensor", "v", v_statrow[1][0])
    p_rsqb = inc("tensor", "p", lambda e: e.matmul(
        rbp[:], sb["onesf"][0:1, :], sb["rsq_row"][0:1, :], start=True, stop=True))
    wait("vector", "p", p_rsqb)
    wait("vector", "dxtq", d_xtq)
    run("vector", lambda e: e.tensor_copy(sb["rsqb"][:], rbp[:]))
    v_negmurs = inc("vector", "v",
                    lambda e: e.tensor_copy(sb["negmurs"][:], sb["nmr_row"][0:1, :]))
    for ct in range(8):
        fn = lambda e, ct=ct: e.tensor_tensor(
            out=sb["xst"][:, ct * NQ:(ct + 1) * NQ],
            in0=sb["xtq"][:, ct * NQ:(ct + 1) * NQ], in1=sb["rsqb"][:], op=OP.mult)
        if ct == 7:
            v_xst = inc("vector", "v", fn)
        else:
            run("vector", fn)

    # ========== TENSOR: v transposes (vp double-buffered) ==========
    p_vt = [0] * JT
    v_vcopy = [0] * JT
    for jt in range(JT):
        pb = vp[jt % 2]
        ch_hi = min(3, ((jt + 1) * 128 - 1 - NUM_NULL) // 512)
        wait("tensor", "s", s_kvevac[ch_hi])
        if jt == JT - 1:
            wait("tensor", "v", v_memset)
        if jt >= 2:
            wait("tensor", "v", v_vcopy[jt - 2])
        p_vt[jt] = inc("tensor", "p", lambda e, pb=pb, jt=jt: e.transpose(
            pb[:], sb["kv"][64:128, jt * 128:(jt + 1) * 128],
            sb["ibf"][64:128, 64:128]))
        wait("vector", "p", p_vt[jt])
        v_vcopy[jt] = inc("vector", "v", lambda e, pb=pb, jt=jt: e.tensor_copy(
            sb["vsb"][:, jt * 65:jt * 65 + 64], pb[:]))
    v_vsb = v_vcopy[JT - 1]

    # ========== TENSOR: q projection (qp double-buffered), head-major evac ===
    wait("tensor", "v", v_xst)
    wait("tensor", "dwq", d_wq)
    p_q = [0] * 8
    v_qtmp = [0] * 8
    for dtile in range(8):
        pb = qp[dtile % 2]
        if dtile >= 2:
            wait("tensor", "v", v_qtmp[dtile - 2])
        for ct in range(8):
            run("tensor", lambda e, pb=pb, dtile=dtile, ct=ct: e.matmul(
                pb[:],
                sb["wq"][:, ct * INNER + dtile * 128:ct * INNER + (dtile + 1) * 128],
                sb["xst"][:, ct * NQ:(ct + 1) * NQ],
                start=(ct == 0), stop=False))
        p_q[dtile] = inc("tensor", "p", lambda e, pb=pb, dtile=dtile: e.matmul(
            pb[:], sb["srow"][0:1, dtile * 128:(dtile + 1) * 128],
            sb["negmurs"][:], start=False, stop=True))
        wait("vector", "p", p_q[dtile])
        # even head (psum rows 0:64) -> qh[p][:, 0:256] directly
        run("vector", lambda e, pb=pb, dtile=dtile: e.tensor_copy(
            qh[dtile][0:64, 0:NQ], pb[0:64, :]))
        slot = dtile % 2
        if dtile >= 2:
            wait("vector", f"dq{dtile - 2}", 16)  # qtmp slot reuse
        v_qtmp[dtile] = inc("vector", "v", lambda e, pb=pb, slot=slot:
                            e.tensor_copy(
                                sb["qtmp"][64:128, slot * NQ:(slot + 1) * NQ],
                                pb[64:128, :]))
        wait("sync", "v", v_qtmp[dtile])
        dma(f"dq{dtile}", qh[dtile][0:64, NQ:2 * NQ],
            sb["qtmp"][64:128, slot * NQ:(slot + 1) * NQ])

    # ========== PAIR LOOP ==========
    v_pre = max(v_vsb, v_qtmp[7])
    p_simc = [[0] * NCK for _ in range(NPAIR)]
    s_exp = [[0] * NCK for _ in range(NPAIR)]
    v_mult = [[0] * NCK for _ in range(NPAIR)]
    p_odone = [0] * NPAIR
    v_oevac = [0] * NPAIR
    p_bcast = [0] * NPAIR
    v_normmult = [0] * NPAIR

    def emit_fill(p, ci):
        # regions alternate by global chunk index: reuse guard is the exp of
        # the chunk two slots earlier, which finished two chunk-periods ago
        g = NCK * p + ci
        ps = SIMREG[g % 2]
        if g >= 2:
            pp, cp = divmod(g - 2, NCK)
            wait("tensor", "s", s_exp[pp][cp])
        if p == 0 and ci == 0:
            wait("tensor", "v", v_pre)      # early psum drained
        if ci == 0:
            wait("tensor", f"dq{p}", 16)
        ch_hi = min(3, ((CHUNKS[ci][-1] + 1) * 128 - 1 - NUM_NULL) // 512)
        if p == 0:
            wait("tensor", "s", s_kvevac[ch_hi])
        base = CH_E1[ci]
        half = CH_LEN[ci] // 2
        # build emission list of bank-safe pieces, then set start on the
        # first piece touching each psum bank and stop on the last (start
        # zeroes the whole 2KB bank; one start/stop pair per bank per group)
        pieces = []
        for jt in CHUNKS[ci]:
            s, w = S_JT[jt], W_JT[jt]
            for h in range(2):
                a0 = h * half + (ECOL1[jt] - base)
                a, b = a0, a0 + w
                while a < b:
                    cut = min(b, (a // 512 + 1) * 512)
                    qa = h * 256 + s + (a - a0)
                    pieces.append([jt, a, cut, qa, qa + (cut - a)])
                    a = cut
        first_in_bank = {}
        last_in_bank = {}
        for pi, (jt, a, b, qa, qb) in enumerate(pieces):
            first_in_bank.setdefault(a // 512, pi)
            last_in_bank[a // 512] = pi
        for pi, (jt, a, b, qa, qb) in enumerate(pieces):
            st = first_in_bank[a // 512] == pi
            sp = last_in_bank[a // 512] == pi
            fn = lambda e, ps=ps, jt=jt, a=a, b=b, qa=qa, qb=qb, st=st, \
                sp=sp: e.matmul(
                    ps[:, a:b], sb["kv"][0:64, jt * 128:(jt + 1) * 128],
                    qh[p][0:64, qa:qb], start=st, stop=sp)
            if pi == len(pieces) - 1:
                p_simc[p][ci] = inc("tensor", "p", fn)
            else:
                run("tensor", fn)

    def emit_ogroup(p):
        eh_ = esb[p % 2]
        if p == 0:
            wait("tensor", "v", v_vsb)
        if p >= 1:
            wait("tensor", "v", v_oevac[p - 1])   # opp reuse
        for ci in range(NCK):
            wait("tensor", "v", v_mult[p][ci])
            for jt in CHUNKS[ci]:
                s, w = S_JT[jt], W_JT[jt]
                for h in range(2):
                    fn = lambda e, jt=jt, s=s, w=w, h=h, eh_=eh_: e.matmul(
                        opp[0:65, h * 256 + s:(h + 1) * 256],
                        sb["vsb"][:, jt * 65:jt * 65 + 65],
                        eh_[:, h * EW1 + ECOL1[jt]:h * EW1 + ECOL1[jt] + w],
                        start=(jt == 0 and h == 0),
                        stop=(jt == JT - 1 and h == 1))
                    if jt == JT - 1 and h == 1:
                        p_odone[p] = inc("tensor", "p", fn)
                    else:
                        run("tensor", fn)

    def emit_bcast(p):
        wait("tensor", "v", v_oevac[p])      # recipflat(p) ready
        if p >= 1:
            wait("tensor", "v", v_normmult[p - 1])   # nrm reuse
        p_bcast[p] = inc("tensor", "p", lambda e, p=p: e.matmul(
            nrm[:], sb["onesbf"][0:1, 0:64],
            sb["recipflat"][0:1, p * 512:(p + 1) * 512], start=True, stop=True))

    def emit_mults(p, ci):
        eh_ = esb[p % 2]
        bh_ = bsb[p % 2]
        wait("vector", "s", s_exp[p][ci])
        if ci == 0:
            wait("vector", f"db{p % 2}", 16 * (p // 2 + 1))
            if p >= 2:
                wait("vector", "p", p_odone[p - 2])   # eh slot reuse
        base = CH_E1[ci]
        half = CH_LEN[ci] // 2
        run("vector", lambda e, half=half, base=base, eh_=eh_, bh_=bh_,
            ci=ci: e.tensor_tensor(
                out=eh_[:, base:base + half], in0=eraw[ci][:, 0:half],
                in1=bh_[:, base:base + half], op=OP.mult))
        v_mult[p][ci] = inc("vector", "v", lambda e, half=half, base=base,
                            eh_=eh_, bh_=bh_, ci=ci: e.tensor_tensor(
            out=eh_[:, EW1 + base:EW1 + base + half],
            in0=eraw[ci][:, half:2 * half],
            in1=bh_[:, EW1 + base:EW1 + base + half], op=OP.mult))

    def emit_oevac(p):
        wait("vector", "p", p_odone[p])
        run("vector", lambda e, p=p: e.tensor_copy(
            sb["oT"][0:64, p * 512:(p + 1) * 512], opp[0:64, :]))
        run("vector", lambda e: e.reciprocal(
            out=sb["rrow"][0:1, :], in_=opp[64:65, :]))
        v_oevac[p] = inc("vector", "v", lambda e, p=p: e.tensor_copy(
            sb["recipflat"][0:1, p * 512:(p + 1) * 512], sb["rrow"][0:1, :]))

    def emit_normmult(p):
        wait("vector", "p", p_bcast[p])
        run("vector", lambda e, p=p: e.tensor_tensor(
            out=sb["oTn"][0:64, p * NQ:(p + 1) * NQ], in0=nrm[0:64, 0:256],
            in1=sb["oT"][0:64, p * 512:p * 512 + 256], op=OP.mult))
        v_normmult[p] = inc("vector", "v", lambda e, p=p: e.tensor_tensor(
            out=sb["oTn_lo"][0:64, p * NQ:(p + 1) * NQ], in0=nrm[0:64, 256:512],
            in1=sb["oT"][0:64, p * 512 + 256:(p + 1) * 512], op=OP.mult))

    def emit_exp(p, ci):
        wait("scalar", "p", p_simc[p][ci])
        if p >= 1:
            wait("scalar", "v", v_mult[p - 1][ci])    # eraw slot reuse
        ps = SIMREG[(NCK * p + ci) % 2]
        ln = CH_LEN[ci]
        s_exp[p][ci] = inc("scalar", "s", lambda e, ps=ps, ln=ln, ci=ci:
                           e.activation(out=eraw[ci][:, 0:ln], in_=ps[:, 0:ln],
                                        func=AF.Exp, bias=sb["zb"][:]))

    for p in range(NPAIR):
        # call order matters only for plan-time counter availability;
        # engine programs are built per-engine in the order emitted below
        emit_fill(p, 0)
        emit_fill(p, 1)
        emit_exp(p, 0)
        emit_exp(p, 1)
        emit_mults(p, 0)
        if p >= 1:
            emit_ogroup(p - 1)
        if p >= 2:
            emit_bcast(p - 2)
        emit_fill(p, 2)
        emit_exp(p, 2)
        emit_mults(p, 1)
        if p >= 1:
            emit_oevac(p - 1)
        if p >= 2:
            emit_normmult(p - 2)
        emit_mults(p, 2)
        # SYNC: bias prefetch for pair p+2; oTn odd-half shuffle for p-2
        if p + 2 < NPAIR:
            wait("sync", "v", v_mult[p][2])
            dma(f"db{(p + 2) % 2}", bsb[(p + 2) % 2][:], d_in["biasp"][p + 2])
        if p >= 2:
            wait("sync", "v", v_normmult[p - 2])
            dma("dn", sb["oTn"][64:128, (p - 2) * NQ:(p - 1) * NQ],
                sb["oTn_lo"][0:64, (p - 2) * NQ:(p - 1) * NQ])

    # epilogue
    emit_ogroup(NPAIR - 1)
    emit_bcast(NPAIR - 2)
    emit_oevac(NPAIR - 1)
    emit_normmult(NPAIR - 2)
    emit_bcast(NPAIR - 1)
    emit_normmult(NPAIR - 1)
    for p in (NPAIR - 2, NPAIR - 1):
        wait("sync", "v", v_normmult[p])
        dma("dn", sb["oTn"][64:128, p * NQ:(p + 1) * NQ],
            sb["oTn_lo"][0:64, p * NQ:(p + 1) * NQ])

    # ========== output projection (accumulators alternate in simA) ==========
    wait("tensor", "v", v_normmult[NPAIR - 1])
    wait("tensor", "dn", 16 * 8)
    wait("tensor", "dw", d_w)
    wait("tensor", "s", s_exp[NPAIR - 1][2])   # sim regions free
    s_outevac = [0] * 8
    OREGS = [simA[:, 0:NQ], simA[:, 512:512 + NQ], simA[:, 1024:1024 + NQ]]
    for et in range(8):
        reg = OREGS[et % 3]
        if et >= 3:
            wait("tensor", "s", s_outevac[et - 3])
        p_wout = 0
        for hdt in range(8):
            fn = lambda e, et=et, hdt=hdt, reg=reg: e.matmul(
                reg, sb["wout"][:, hdt * DIM + et * 128:hdt * DIM + (et + 1) * 128],
                sb["oTn"][:, hdt * NQ:(hdt + 1) * NQ],
                start=(hdt == 0), stop=(hdt == 7))
            if hdt == 7:
                p_wout = inc("tensor", "p", fn)
            else:
                run("tensor", fn)
        wait("scalar", "p", p_wout)
        oslot = et % 3
        if et >= 3:
            wait("scalar", f"do{oslot}", 16 * (et // 3))  # outsb slot reuse
        s_outevac[et] = inc("scalar", "s", lambda e, oslot=oslot, reg=reg:
                            e.activation(
                                out=sb["outsb"][:, oslot * NQ:(oslot + 1) * NQ],
                                in_=reg, func=AF.Copy))
        wait("sync", "s", s_outevac[et])
        dma(f"do{oslot}", out_d[et * 128:(et + 1) * 128, :],
            sb["outsb"][:, oslot * NQ:(oslot + 1) * NQ])

    # ========== emit ==========
    from contextlib import ExitStack as _ES
    semctx = _ES()
    for k in ("p", "v", "s") + DSEMS:
        SEM[k] = semctx.enter_context(nc.semaphore(f"sem_{k}"))
    with semctx:
        with nc.Block() as block:
            @block.sync
            def _(e):
                for fn in plan["sync"]:
                    fn(e)

            @block.tensor
            def _(e):
                for fn in plan["tensor"]:
                    fn(e)

            @block.vector
            def _(e):
                for fn in plan["vector"]:
                    fn(e)

            @block.scalar
            def _(e):
                for fn in plan["scalar"]:
                    fn(e)
    ctx.close()
    return nc


def _prep_inputs(x, attn_bias, Wq, Wkv, null_kv, Wout, gamma, mask):
    from ml_dtypes import bfloat16
    x = np.asarray(x, np.float32)[0]            # [N, DIM]
    attn_bias = np.asarray(attn_bias, np.float32)[0]  # [H, N, N]
    Wq = np.asarray(Wq, np.float32)
    Wkv = np.asarray(Wkv, np.float32)
    null_kv = np.asarray(null_kv, np.float32)
    Wout = np.asarray(Wout, np.float32)
    gamma = np.asarray(gamma, np.float32)
    mask = np.asarray(mask, bool)[0]            # [N]

    scale = DIM_HEAD ** -0.5
    wq_eff = (gamma[:, None] * Wq * scale).astype(np.float32)
    srow = wq_eff.sum(axis=0, keepdims=True)
    xt = np.ascontiguousarray(x.T)
    nkvt = np.zeros((128, NUM_NULL), np.float32)
    nkvt[0:DIM_HEAD, :] = null_kv[0].T
    nkvt[64:64 + DIM_HEAD, :] = null_kv[1].T
    I128 = np.eye(128, dtype=np.float32)
    ones = np.ones((1, 128), np.float32)

    jpad = np.arange(JPAD)
    jvalid = np.zeros(JPAD, bool)
    jvalid[:NUM_NULL] = True
    jvalid[NUM_NULL:NUM_NULL + N] = mask
    key_of_j = jpad - NUM_NULL

    in_maps = []
    idx_all = []
    for c in range(NCORES):
        idx = np.concatenate([np.arange(c, 1024, 8), np.arange(1024 + c, 2048, 8)])
        idx_all.append(idx)
        allow = jvalid[None, :] & (key_of_j[None, :] <= idx[:, None])  # [NQ, JPAD]
        allow[:, :NUM_NULL] = True
        ab = np.zeros((HEADS, JPAD, NQ), np.float32)
        ab[:, NUM_NULL:NUM_NULL + N, :] = attn_bias[:, idx, :].transpose(0, 2, 1)
        bt = np.where(allow.T[None], ab, MASK_VAL)
        ebt = np.exp(bt)                 # exp(bias); exactly 0 where masked
        # pack per head-PAIR, trimmed per-tile: [h0 (w) | h1 (w)] at ECOL[jt]
        pk = np.empty((NPAIR, 128, EW), np.float32)
        for jt in range(JT):
            s, w = S_JT[jt], W_JT[jt]
            c0 = ECOL1[jt]
            tile = ebt[:, jt * 128:(jt + 1) * 128, s:256]     # [H, 128, w]
            pk[:, :, c0:c0 + w] = tile[0::2]
            pk[:, :, EW1 + c0:EW1 + c0 + w] = tile[1::2]
        in_maps.append({
            "xq": np.ascontiguousarray(
                x[idx].reshape(2, 128, DIM).transpose(1, 0, 2).reshape(128, -1)),
            "xtq": np.ascontiguousarray(
                xt[:, idx].reshape(8, 128, NQ).transpose(1, 0, 2).reshape(128, -1)),
            "xt": np.ascontiguousarray(
                xt.reshape(8, 128, 4, 512).transpose(1, 2, 0, 3)
                .reshape(128, -1)).astype(bfloat16),
            "wq": np.ascontiguousarray(
                wq_eff.reshape(8, 128, INNER).transpose(1, 0, 2)
                .reshape(128, -1)).astype(bfloat16),
            "srow": srow.astype(bfloat16),
            "wkv": np.ascontiguousarray(
                Wkv.reshape(8, 128, 128).transpose(1, 0, 2)
                .reshape(128, -1)).astype(bfloat16),
            "nkvt": nkvt.astype(bfloat16),
            "wout": np.ascontiguousarray(
                Wout.reshape(8, 128, DIM).transpose(1, 0, 2)
                .reshape(128, -1)).astype(bfloat16),
            "ibf": I128.astype(bfloat16),
            "if32": I128,
            "onesbf": ones.astype(bfloat16),
            "onesf": ones,
            "biasp": pk.astype(bfloat16),
        })
    return in_maps, idx_all


def _run(inputs, trace=False):
    from concourse.bass_utils import run_bass_kernel_spmd
    if "nc" not in _CACHE:
        _CACHE["nc"] = _build_graph()
    nc = _CACHE["nc"]
    in_maps, idx_all = _prep_inputs(**inputs)
    res = run_bass_kernel_spmd(nc, in_maps, list(range(NCORES)), trace=trace)
    out = np.zeros((B, N, DIM), np.float32)
    for c in range(NCORES):
        out[0, idx_all[c], :] = res.results[c]["out"].T
    return out, res


def kernel(**inputs):
    out, _ = _run(inputs, trace=False)
    return out
